# revision 20
# baseline (speedup 1.0000x reference)
"""GAT (2-layer, 4-head) Trainium2 kernel over 8 NeuronCores — v2.

Strategy:
  * Edges sorted by dst, dst-range partitioned across the 8 cores (each core
    owns N/8 node rows and fully computes their output -> no output
    all-reduce, softmax stats stay core-local).
  * Per layer: node GEMM is data-parallel over the owned node range, the
    augmented node table (h | ones | s_src | s_dst) is AllGathered to every
    core's HBM, then the edge phase gathers h[src] rows with dma_gather and
    performs the segment softmax + weighted scatter-add as one-hot matmuls
    accumulated in PSUM.
  * v2: the per-edge s_dst values come from a transposed one-hot matmul
    against the tile's own s_dst vector (built via a rank-1 PE broadcast of
    the host-known dstrel row + DVE is_equal) instead of a second dma_gather;
    per-(tile,half) slot counts are exact (max over cores) instead of a
    global max; gathers round-robin over 4 SWDGE queues; the classifier is
    fused into layer 2's edge phase.
  * int16 gather indices cap at 32767, so the node table is addressed as two
    halves (rows < VH and the rest) with per-tile A/B edge slot groups.
"""

import sys

if "/opt/trn_rl_repo" not in sys.path:
    sys.path.insert(0, "/opt/trn_rl_repo")

import ml_dtypes
import numpy as np

import concourse.bacc as bacc
import concourse.bass as bass
import concourse.mybir as mybir
import concourse.tile as tile
from concourse.bass_utils import run_bass_kernel_spmd

BF16 = mybir.dt.bfloat16
F32 = mybir.dt.float32
I16 = mybir.dt.int16

NCORES = 8
P = 128

CFG = dict(
    N=50000,
    E=500000,
    F=256,      # feature width (in = hid = 256)
    H=4,
    DH=64,
    OUT=64,
    ROW=256,    # bf16 row length of node table (512B: h only)
    SLOTMAX=28,  # max 128-edge slots per gather group
    NQ=4,       # SWDGE queues for gathers
)


# --------------------------------------------------------------------------
# host-side preparation
# --------------------------------------------------------------------------

def _head_matrix(a):
    """[H, DH] -> block diagonal [F, H] so that s = h @ A."""
    H, DH = np.asarray(a).shape
    A = np.zeros((H * DH, H), np.float64)
    for h in range(H):
        A[h * DH:(h + 1) * DH, h] = np.asarray(a, np.float64)[h]
    return A


def _wfull(W, a_src, a_dst):
    """[W^T | zeros | W^T@Asrc | W^T@Adst] as [F, 268] bf16."""
    W = np.asarray(W, np.float64)
    F = W.shape[1]
    Wt = W.T
    Bs = Wt @ _head_matrix(a_src)
    Bd = Wt @ _head_matrix(a_dst)
    out = np.zeros((F, 268), np.float64)
    out[:, :W.shape[0]] = Wt
    out[:, 260:264] = Bs
    out[:, 264:268] = Bd
    return out.astype(ml_dtypes.bfloat16)


def _bn_consts(gamma, beta, mean, var, eps=1e-5):
    gamma = np.asarray(gamma, np.float64)
    beta = np.asarray(beta, np.float64)
    mean = np.asarray(mean, np.float64)
    var = np.asarray(var, np.float64)
    g = gamma / np.sqrt(var + eps)
    b = beta - mean * g
    F = gamma.shape[0]
    return (
        np.ascontiguousarray(g.reshape(F // P, P).T.astype(np.float32)),
        np.ascontiguousarray(b.reshape(F // P, P).T.astype(np.float32)),
    )


def _wrap_idx(flat):
    """int16 position array -> dma_gather wrapped layout [128, len//16]."""
    n = len(flat)
    assert n % 16 == 0
    w = np.zeros((P, n // 16), np.int16)
    w[:16, :] = np.asarray(flat, np.int16).reshape(-1, 16).T
    w[16:, :] = np.tile(w[:16, :], (7, 1))
    return w


def _asrc_rep(a_src):
    """a_src [H, DH] -> [P, 268] bf16 row-replicated flat vector."""
    flat = np.zeros(268, np.float32)
    flat[:256] = np.asarray(a_src, np.float32).reshape(-1)
    return np.ascontiguousarray(
        np.tile(flat[None, :], (P, 1))).astype(ml_dtypes.bfloat16)


def balance_perm(cfg, edge_index):
    """Permute nodes within each core's range so per-tile (A,B) in-degree
    sums are balanced -> fewer 128-edge gather slots. Returns pos[id]."""
    N = cfg["N"]
    NB = ((N + NCORES - 1) // NCORES + P - 1) // P * P
    NT = NB // P
    RA = (NT // 2) * P
    NPAD = NB * NCORES
    src = np.asarray(edge_index[0], np.int64)
    dst = np.asarray(edge_index[1], np.int64)
    TSPLIT = NT // 2
    # nodes never change half (A = local < RA), so the (inA, inB) in-degree
    # labels stay exact and one packing pass is sufficient
    a_lab = (src % NB) < RA
    inA = np.bincount(dst[a_lab], minlength=NPAD).astype(np.int64)
    inB = np.bincount(dst[~a_lab], minlength=NPAD).astype(np.int64)
    base = 5 * P
    # fixed overflow-tile indices shared by all cores (per-tile slot count
    # is a max over cores): per half-group, A overflow at its low tiles,
    # B overflow at its high tiles
    capA_t = np.full(NT, base, np.int64)
    capB_t = np.full(NT, base, np.int64)
    for t0, nbin in ((0, TSPLIT), (TSPLIT, NT - TSPLIT)):
        needA = needB = 0
        for k in range(NCORES):
            ids = np.arange(k * NB + t0 * P, k * NB + (t0 + nbin) * P)
            needA = max(needA, (inA[ids].sum() - nbin * base + P - 1) // P)
            needB = max(needB, (inB[ids].sum() - nbin * base + P - 1) // P)
        nExA = int(max(needA, 0)) + 2
        nExB = int(max(needB, 0)) + 2
        capA_t[t0:t0 + nExA] += P
        capB_t[t0 + nbin - nExB:t0 + nbin] += P
    pos = np.empty(NPAD, np.int64)
    for k in range(NCORES):
        for t0, nbin in ((0, TSPLIT), (TSPLIT, NT - TSPLIT)):
            lo = k * NB + t0 * P
            ids = np.arange(lo, lo + nbin * P)
            capA = capA_t[t0:t0 + nbin]
            capB = capB_t[t0:t0 + nbin]
            tot = inA[ids] + inB[ids]
            order = np.argsort(-tot, kind="stable")
            sumA = np.zeros(nbin)
            sumB = np.zeros(nbin)
            cnt = np.zeros(nbin)
            lane = np.zeros(nbin, np.int64)
            fA = capA.astype(np.float64)
            fB = capB.astype(np.float64)
            for j in order:
                a, b = inA[ids[j]], inB[ids[j]]
                load = np.maximum((sumA + a) / fA, (sumB + b) / fB)
                load = np.maximum(load, (cnt + 1) / P) + (cnt >= P) * 1e9
                t = int(np.argmin(load))
                sumA[t] += a
                sumB[t] += b
                cnt[t] += 1
                pos[ids[j]] = lo + t * P + lane[t]
                lane[t] += 1
    return pos


def prep_edges(cfg, edge_index):
    """Sort/partition edges; exact per-(tile,half) slot counts (max over
    cores), greedy tile groups bounded by SLOTMAX slots."""
    N, SLOTMAX = cfg["N"], cfg["SLOTMAX"]
    NB = ((N + NCORES - 1) // NCORES + P - 1) // P * P
    NT = NB // P
    SPLIT2 = cfg.get("SPLIT2", 1)
    src = np.asarray(edge_index[0], np.int64)
    dst = np.asarray(edge_index[1], np.int64)
    core = dst // NB
    tilein = (dst % NB) // P
    if SPLIT2:
        # split the shared table at tile TSPLIT: A = local rows < RA, B =
        # rest; both halves must index within int16 across all cores
        TSPLIT = NT // 2
        RA = TSPLIT * P
        RB = NB - RA
        assert NCORES * RA <= 32768 and NCORES * RB <= 32768
        cfg["TSPLIT"] = TSPLIT
        half = ((src % NB) >= RA).astype(np.int64)
    else:
        VH = min(32768, NB * NCORES // 2)
        cfg["VH"] = VH
        cfg["TSPLIT"] = NT
        half = (src >= VH).astype(np.int64)
    order = np.lexsort((src, half, tilein, core))
    sc, tc, hc = core[order], tilein[order], half[order]
    ss, ds = src[order], dst[order]
    if SPLIT2:
        # remap src to half-table indices
        s_core = ss // NB
        s_loc = ss % NB
        ss = np.where(s_loc < RA, s_core * RA + s_loc,
                      s_core * RB + (s_loc - RA))
    else:
        ss = np.where(ss < VH, ss, ss - VH)
    key = (sc * NT + tc) * 2 + hc
    bounds = np.searchsorted(key, np.arange(NCORES * NT * 2 + 1))
    lists = {}
    for k in range(NCORES):
        for t in range(NT):
            for h in (0, 1):
                j = (k * NT + t) * 2 + h
                i0, i1 = bounds[j], bounds[j + 1]
                lists[(k, t, h)] = (ss[i0:i1], ds[i0:i1] % P)
    # exact slot counts per (tile, half): max over cores
    KA = [max(max((len(lists[(k, t, 0)][0]) for k in range(NCORES))), 1)
          for t in range(NT)]
    KB = [max(max((len(lists[(k, t, 1)][0]) for k in range(NCORES))), 1)
          for t in range(NT)]
    KA = [(c + P - 1) // P for c in KA]
    KB = [(c + P - 1) // P for c in KB]

    # greedy groups of consecutive tiles, <= SLOTMAX slots each
    groups = []  # list of (t0, gn, SA, SB)
    t0 = 0
    while t0 < NT:
        gn = 1
        sa, sb = KA[t0], KB[t0]
        while (t0 + gn < NT
               and sa + sb + KA[t0 + gn] + KB[t0 + gn] <= SLOTMAX):
            sa += KA[t0 + gn]
            sb += KB[t0 + gn]
            gn += 1
        groups.append((t0, gn, sa, sb))
        t0 += gn
    TOT = sum(sa + sb for _, _, sa, sb in groups)

    per_core = []
    for k in range(NCORES):
        idxa_cols, idxb_cols, drr_cols = [], [], []
        dstrel = np.full((P, TOT), 128.0, np.float32)
        soff = 0
        for (g0, gn, sa, sb) in groups:
            fa = np.zeros(sa * P, np.int16)
            fb = np.zeros(sb * P, np.int16)
            drow = np.full((sa + sb) * P, 128.0, np.float32)
            aoff = 0
            boff = sa
            for tl in range(gn):
                t = g0 + tl
                for h in (0, 1):
                    s_arr, r_arr = lists[(k, t, h)]
                    n = len(s_arr)
                    if h == 0:
                        base = aoff * P
                        fa[base:base + n] = s_arr.astype(np.int16)
                        slot0 = aoff
                        aoff += KA[t]
                    else:
                        base = (boff - sa) * P
                        fb[base:base + n] = s_arr.astype(np.int16)
                        slot0 = boff
                        boff += KB[t]
                    for i in range(n):
                        dstrel[i % P, soff + slot0 + i // P] = r_arr[i]
                        drow[(slot0 + i // P) * P + i % P] = r_arr[i]
            idxa_cols.append(_wrap_idx(fa))
            idxb_cols.append(_wrap_idx(fb))
            drr_cols.append(drow)
            soff += sa + sb
        per_core.append(dict(
            idxa=np.concatenate(idxa_cols, axis=1),
            idxb=np.concatenate(idxb_cols, axis=1),
            dstrel=dstrel.astype(ml_dtypes.bfloat16),
            dstrelr=np.concatenate(drr_cols)[None, :].astype(ml_dtypes.bfloat16),
        ))
    return groups, KA, KB, TOT, per_core, NB, NT


# --------------------------------------------------------------------------
# device kernel
# --------------------------------------------------------------------------

def apx(base_ap, pairs, extra_offset=0):
    return bass.AP(base_ap.tensor, base_ap.offset + extra_offset,
                   [list(p) for p in pairs])


def build_kernel(cfg, groups, KA, KB, TOT, NB, NT):
    F, H, DH, OUT = cfg["F"], cfg["H"], cfg["DH"], cfg["OUT"]
    ROW, TSPLIT = cfg["ROW"], cfg["TSPLIT"]
    SLOTMAX, NQ = cfg["SLOTMAX"], cfg["NQ"]
    SPLIT2 = cfg.get("SPLIT2", 1)
    FC = F // P
    NPAD = NB * NCORES
    RA = TSPLIT * P
    RB = NB - RA
    AluOp = mybir.AluOpType
    Act = mybir.ActivationFunctionType
    ABL = cfg.get("ABL", 5)

    nc = bacc.Bacc("TRN2", target_bir_lowering=False, debug=False,
                   num_devices=NCORES, num_swdge_queues=NQ)

    # ---- I/O ----
    xt_in = nc.declare_dram_parameter("xt", [FC, P, NB], BF16, isOutput=False)
    wf_in = [nc.declare_dram_parameter(f"wfull{l + 1}", [FC, P, 268], BF16,
                                       isOutput=False) for l in range(2)]
    wct_in = nc.declare_dram_parameter("wct", [FC, P, OUT], BF16, isOutput=False)
    gv_in = [nc.declare_dram_parameter(f"gvec{l + 1}", [P, FC], F32,
                                       isOutput=False) for l in range(2)]
    bv_in = [nc.declare_dram_parameter(f"bvec{l + 1}", [P, FC], F32,
                                       isOutput=False) for l in range(2)]
    bc_in = nc.declare_dram_parameter("bc_rep", [P, OUT], F32, isOutput=False)
    SA_tot = sum(sa for _, _, sa, _ in groups)
    SB_tot = sum(sb for _, _, _, sb in groups)
    idxa_in = nc.declare_dram_parameter("idxa", [P, SA_tot * 8], I16,
                                        isOutput=False)
    idxb_in = nc.declare_dram_parameter("idxb", [P, SB_tot * 8], I16,
                                        isOutput=False)
    ident_in = nc.declare_dram_parameter("ident", [P, P], F32, isOutput=False)
    asrc_in = [nc.declare_dram_parameter(f"asrcr{l + 1}", [P, 268], BF16,
                                         isOutput=False) for l in range(2)]
    iota_in = nc.declare_dram_parameter("iotarow", [P, P], BF16, isOutput=False)
    iotac_in = nc.declare_dram_parameter("iotacol", [P, P], F32, isOutput=False)
    ones1_in = nc.declare_dram_parameter("ones1", [1, P], BF16, isOutput=False)
    dstrel_in = nc.declare_dram_parameter("dstrel", [P, TOT], BF16,
                                          isOutput=False)
    drr_in = nc.declare_dram_parameter("dstrelr", [1, TOT * P], BF16,
                                       isOutput=False)
    out_ext = nc.declare_dram_parameter("out", [NB, OUT], F32, isOutput=True)

    if SPLIT2:
        haug_ownA = [nc.dram_tensor(f"haug_ownA{l}", [RA, ROW], BF16)
                     for l in (0, 1)]
        haug_ownB = [nc.dram_tensor(f"haug_ownB{l}", [RB, ROW], BF16)
                     for l in (0, 1)]
        haug_allA = [nc.dram_tensor(f"haug_allA{l}", [NCORES, RA, ROW], BF16,
                                    addr_space="Shared") for l in (0, 1)]
        haug_allB = [nc.dram_tensor(f"haug_allB{l}", [NCORES, RB, ROW], BF16,
                                    addr_space="Shared") for l in (0, 1)]
    else:
        haug_own = [nc.dram_tensor(f"haug_own{l}", [NB, ROW], BF16)
                    for l in (0, 1)]
        haug_all = [nc.dram_tensor(f"haug_all{l}", [NCORES, NB, ROW], BF16,
                                   addr_space="Shared") for l in (0, 1)]


    with tile.TileContext(nc) as tc:
        with (
            tc.tile_pool(name="const", bufs=1) as cpool,
            tc.tile_pool(name="persist", bufs=1) as ppool,
            tc.tile_pool(name="work", bufs=2) as wpool,
            tc.tile_pool(name="works", bufs=4) as spool,
            tc.tile_pool(name="gath", bufs=2) as gpool,
            tc.tile_pool(name="psmm", bufs=2, space="PSUM") as pspool,
            tc.tile_pool(name="psacc", bufs=2, space="PSUM") as accpool,
            tc.tile_pool(name="pssd", bufs=2, space="PSUM") as sdpool,
        ):
            # ---- constants ----
            ident = cpool.tile([P, P], F32)
            nc.sync.dma_start(out=ident[:, :], in_=ident_in[:, :])
            asrc_sb = [cpool.tile([P, 268], BF16, tag=f"asr{l}",
                                  name=f"asr{l}") for l in range(2)]
            for l in range(2):
                nc.sync.dma_start(out=asrc_sb[l][:, :], in_=asrc_in[l][:, :])
            iota_bf = cpool.tile([P, P], BF16)
            nc.sync.dma_start(out=iota_bf[:, :], in_=iota_in[:, :])
            iotac = cpool.tile([P, P], F32)
            nc.sync.dma_start(out=iotac[:, :], in_=iotac_in[:, :])
            ones1 = cpool.tile([1, P], BF16)
            nc.sync.dma_start(out=ones1[:, :], in_=ones1_in[:, :])
            wf_sb = [cpool.tile([P, FC, 268], BF16, tag=f"wf{l}", name=f"wf{l}")
                     for l in range(2)]
            for l in range(2):
                nc.sync.dma_start(out=wf_sb[l][:, :, :],
                                  in_=wf_in[l].rearrange("c p n -> p c n"))
            wct_sb = cpool.tile([P, FC, OUT], BF16)
            nc.sync.dma_start(out=wct_sb[:, :, :],
                              in_=wct_in.rearrange("c p n -> p c n"))
            gv_sb = [cpool.tile([P, FC], F32, tag=f"gv{l}", name=f"gv{l}")
                     for l in range(2)]
            bv_sb = [cpool.tile([P, FC], F32, tag=f"bv{l}", name=f"bv{l}")
                     for l in range(2)]
            for l in range(2):
                nc.sync.dma_start(out=gv_sb[l][:, :], in_=gv_in[l][:, :])
                nc.sync.dma_start(out=bv_sb[l][:, :], in_=bv_in[l][:, :])
            bc_sb = cpool.tile([P, OUT], F32)
            nc.sync.dma_start(out=bc_sb[:, :], in_=bc_in[:, :])
            dstrel_sb = cpool.tile([P, TOT], BF16)
            nc.sync.dma_start(out=dstrel_sb[:, :], in_=dstrel_in[:, :])
            SA_tot_ = sum(sa for _, _, sa, _ in groups)
            SB_tot_ = sum(sb for _, _, _, sb in groups)
            idxa_sb = cpool.tile([P, SA_tot_ * 8], I16)
            nc.sync.dma_start(out=idxa_sb[:, :], in_=idxa_in[:, :])
            idxb_sb = cpool.tile([P, SB_tot_ * 8], I16)
            nc.sync.dma_start(out=idxb_sb[:, :], in_=idxb_in[:, :])

            # buf0 = x (layer-1 input, never overwritten)
            # buf1 = layer-1 edge output (layer-2 input)
            xt_sb = [ppool.tile([P, FC, NB], BF16, tag=f"xt{l}", name=f"xt{l}")
                     for l in range(2)]
            nc.sync.dma_start(out=xt_sb[0][:, :, :],
                              in_=xt_in.rearrange("c p n -> p c n"))
            sdst_sb = ppool.tile([P, NT * H], BF16)

            for rep_ in range(cfg.get("REPEAT", 1)):
                for layer in (0, 1):
                    wfl = wf_sb[layer]
                    xt = xt_sb[layer]

                    # ---- node GEMM -> haug_own halves + local s_dst ----
                    for t in range(NT):
                        ps = pspool.tile([P, 512], F32, tag="mm")
                        for kc in range(FC):
                            nc.tensor.matmul(
                                ps[:, 0:268],
                                lhsT=xt[:, kc, t * P:(t + 1) * P],
                                rhs=wfl[:, kc, :],
                                start=(kc == 0), stop=(kc == FC - 1),
                            )
                        stg = spool.tile([P, ROW], BF16, tag="gemmout")
                        nc.scalar.copy(stg[:, :], ps[:, 0:ROW])
                        if not SPLIT2:
                            nc.sync.dma_start(
                                out=haug_own[layer][t * P:(t + 1) * P, :],
                                in_=stg[:, :])
                        elif t < TSPLIT:
                            nc.sync.dma_start(
                                out=haug_ownA[layer][t * P:(t + 1) * P, :],
                                in_=stg[:, :])
                        else:
                            t2 = t - TSPLIT
                            nc.sync.dma_start(
                                out=haug_ownB[layer][t2 * P:(t2 + 1) * P, :],
                                in_=stg[:, :])
                        nc.vector.tensor_copy(sdst_sb[:, t * H:(t + 1) * H],
                                              ps[:, 264:268])
                        if SPLIT2 and t == TSPLIT - 1:
                            # ---- share first half of the node table ----
                            if ABL == 4:
                                nc.sync.dma_start(
                                    out=haug_allA[layer][0, :, :],
                                    in_=haug_ownA[layer][:, :])
                            else:
                                nc.gpsimd.collective_compute(
                                    "AllGather", AluOp.bypass,
                                    replica_groups=[list(range(NCORES))],
                                    ins=[haug_ownA[layer][:, :]],
                                    outs=[haug_allA[layer][:, :, :]],
                                )
                    # ---- share second half (or, SPLIT2=0, everything) ----
                    if SPLIT2:
                        c_in, c_out = haug_ownB[layer], haug_allB[layer]
                    else:
                        c_in, c_out = haug_own[layer], haug_all[layer]
                    if ABL == 4:
                        nc.sync.dma_start(out=c_out[0, :, :], in_=c_in[:, :])
                    else:
                        nc.gpsimd.collective_compute(
                            "AllGather", AluOp.bypass,
                            replica_groups=[list(range(NCORES))],
                            ins=[c_in[:, :]],
                            outs=[c_out[:, :, :]],
                        )
                    if SPLIT2:
                        hflatA = haug_allA[layer].rearrange("c n d -> (c n) d")
                        hflatB = haug_allB[layer].rearrange("c n d -> (c n) d")
                    else:
                        hfl = haug_all[layer].rearrange("c n d -> (c n) d")
                        VH = cfg["VH"]
                        hflatA = hfl[0:VH, :]
                        hflatB = hfl[VH:NPAD, :]
                    if ABL == 1:
                        if layer == 0:
                            nc.vector.memset(xt_sb[1][:, :, :], 0.1)
                        continue

                    # ---- edge phase ----
                    offa = offb = offs = 0
                    for gi, (g0, gn, sa, sb) in enumerate(groups):
                        S = sa + sb
                        drr_t = wpool.tile([1, SLOTMAX * P], BF16, tag="drr")
                        nc.sync.dma_start(
                            out=drr_t[:, 0:S * P],
                            in_=drr_in[:, offs * P:(offs + S) * P])

                        gat = gpool.tile([P, SLOTMAX, ROW], BF16, tag="gat")
                        if ABL == 2:
                            nc.vector.memset(gat[:, :, :], 0.05)
                            nc.vector.tensor_copy(gat[:, 0:1, 0:8],
                                                  idxa_sb[:, 0:8])
                        else:
                            nc.gpsimd.dma_gather(
                                out_ap=gat[:, 0:sa, :], in_ap=hflatA[:, :],
                                idxs_ap=idxa_sb[:, offa:offa + sa * 8],
                                num_idxs=sa * P,
                                num_idxs_reg=sa * P, elem_size=ROW,
                                single_packet=False,
                                queue_num=(2 * gi) % NQ)
                            nc.gpsimd.dma_gather(
                                out_ap=gat[:, sa:S, :], in_ap=hflatB[:, :],
                                idxs_ap=idxb_sb[:, offb:offb + sb * 8],
                                num_idxs=sb * P,
                                num_idxs_reg=sb * P, elem_size=ROW,
                                single_packet=False,
                                queue_num=(2 * gi + 1) % NQ)
                        offa += sa * 8
                        offb += sb * 8

                        # transposed one-hot: ohT[n, s, e] = (n == dstrel[s,e])
                        ohT = wpool.tile([P, SLOTMAX, P], BF16, tag="ohT")
                        nch = (S + 3) // 4 if ABL != 10 else 0
                        for c in range(nch):
                            cols = min(4, S - 4 * c) * P
                            bc_ps = pspool.tile([P, 512], F32, tag="mm",
                                                name="bc_ps")
                            nc.tensor.matmul(
                                bc_ps[:, 0:cols],
                                lhsT=ones1[:, :],
                                rhs=drr_t[:, 4 * c * P:4 * c * P + cols],
                                start=True, stop=True)
                            iotac_ap = iotac[:, :]
                            nc.vector.tensor_tensor(
                                out=ohT[:, 4 * c:4 * c + cols // P, :],
                                in0=apx(iotac_ap,
                                        [iotac_ap.ap[0], [0, cols // P],
                                         [1, P]]),
                                in1=apx(bc_ps[:, :],
                                        [bc_ps[:, :].ap[0], [P, cols // P],
                                         [1, P]]),
                                op=AluOp.is_equal)

                        # per-edge s_dst via ohT matmul against own s_dst
                        sdacc = sdpool.tile([P, SLOTMAX, H], F32, tag="sdacc")
                        if ABL != 10:
                            sl = 0
                            for h_ in (0, 1):
                                for tl in range(gn):
                                    t = g0 + tl
                                    for _ in range(KA[t] if h_ == 0 else KB[t]):
                                        nc.tensor.matmul(
                                            sdacc[:, sl, :],
                                            lhsT=ohT[:, sl, :],
                                            rhs=sdst_sb[:, t * H:(t + 1) * H],
                                            start=True, stop=True)
                                        sl += 1

                        # s_src per edge from gathered h; hts is scratch
                        hts = wpool.tile([P, SLOTMAX, 260], BF16, tag="hts")
                        asl = asrc_sb[layer][:, 0:256]
                        nc.vector.tensor_tensor(
                            out=hts[:, 0:S, 0:256],
                            in0=gat[:, 0:S, :],
                            in1=apx(asl, [asl.ap[0], [0, S], [1, 256]]),
                            op=AluOp.mult)
                        ssrc = spool.tile([P, SLOTMAX, H], F32, tag="ssrc")
                        nc.vector.tensor_reduce(
                            out=ssrc[:, 0:S, :],
                            in_=hts[:, 0:S, 0:256].rearrange(
                                "p s (h d) -> p s h d", h=H),
                            axis=mybir.AxisListType.X, op=AluOp.add)
                        # e = lrelu(ssrc + sdst); w = exp(e)
                        ef = spool.tile([P, SLOTMAX, H], F32, tag="ef")
                        if ABL == 10:
                            nc.vector.tensor_scalar(
                                out=ef[:, 0:S, :], in0=ssrc[:, 0:S, :],
                                scalar1=1.0, scalar2=None, op0=AluOp.mult)
                        else:
                            nc.vector.tensor_tensor(
                                out=ef[:, 0:S, :], in0=ssrc[:, 0:S, :],
                                in1=sdacc[:, 0:S, :], op=AluOp.add)
                        efs = spool.tile([P, SLOTMAX, H], F32, tag="efs")
                        nc.vector.tensor_scalar(
                            out=efs[:, 0:S, :], in0=ef[:, 0:S, :], scalar1=0.2,
                            scalar2=None, op0=AluOp.mult)
                        nc.vector.tensor_tensor(
                            out=ef[:, 0:S, :], in0=ef[:, 0:S, :],
                            in1=efs[:, 0:S, :], op=AluOp.max)
                        wexp = spool.tile([P, SLOTMAX, H], BF16, tag="wexp")
                        nc.scalar.activation(wexp[:, 0:S, :], ef[:, 0:S, :],
                                             Act.Exp)

                        # one-hot [128e, S, 128n]
                        oh = wpool.tile([P, SLOTMAX, P], BF16, tag="oh")
                        dr = dstrel_sb[:, offs:offs + S]
                        iota_ap = iota_bf[:, :]
                        nc.vector.tensor_tensor(
                            out=oh[:, 0:S, :],
                            in0=apx(iota_ap, [iota_ap.ap[0], [0, S], [1, P]]),
                            in1=dr.to_broadcast([P, S, P]),
                            op=AluOp.is_equal)

                        # scale gathered rows by w; cols 256:260 = w itself
                        if ABL == 12:
                            nc.vector.tensor_copy(hts[:, 0:1, :],
                                                  gat[:, 0:1, 0:260])
                        else:
                            nc.vector.tensor_tensor(
                                out=hts[:, 0:S, 0:256].rearrange(
                                    "p s (h d) -> p s h d", h=H),
                                in0=gat[:, 0:S, 0:256].rearrange(
                                    "p s (h d) -> p s h d", h=H),
                                in1=wexp[:, 0:S, :].to_broadcast([P, S, H, DH]),
                                op=AluOp.mult)
                            nc.vector.tensor_copy(hts[:, 0:S, 256:260],
                                                  wexp[:, 0:S, :])

                        # scatter + normalize + BN/ELU per tile
                        aoff = 0
                        boff = sa
                        for tl in range(gn):
                            t = g0 + tl
                            slots = ([aoff + s for s in range(KA[t])] +
                                     [boff + s for s in range(KB[t])])
                            aoff += KA[t]
                            boff += KB[t]
                            if ABL == 11:
                                slots = slots[:1]
                            K_t = len(slots)
                            acc = accpool.tile([P, 260], F32, tag="acc")
                            rsrc = gat if ABL == 12 else hts
                            for j, slx in enumerate(slots):
                                nc.tensor.matmul(
                                    acc[:, :],
                                    lhsT=oh[:, slx, :],
                                    rhs=rsrc[:, slx, 0:260],
                                    start=(j == 0), stop=(j == K_t - 1),
                                )
                            tmax = spool.tile([P, H], F32, tag="tmax")
                            nc.vector.tensor_scalar(
                                out=tmax[:, :], in0=acc[:, 256:260],
                                scalar1=1e-9, scalar2=None, op0=AluOp.max)
                            rec = spool.tile([P, H], F32, tag="rec")
                            nc.vector.reciprocal(rec[:, :], tmax[:, :])
                            zsb = spool.tile([P, F], F32, tag="zsb")
                            nc.vector.tensor_tensor(
                                out=zsb[:, :].rearrange("p (h d) -> p h d",
                                                        h=H),
                                in0=acc[:, 0:256].rearrange("p (h d) -> p h d",
                                                            h=H),
                                in1=rec[:, :].to_broadcast([P, H, DH]),
                                op=AluOp.mult)
                            # transpose + BN + ELU per feature chunk
                            cls_in = spool.tile([P, FC, P], BF16, tag="clsin")
                            for fc in range(FC):
                                pst = pspool.tile([P, P], F32, tag="ptr")
                                nc.tensor.transpose(
                                    pst[:, :], zsb[:, fc * P:(fc + 1) * P],
                                    ident[:, :])
                                ybn = spool.tile([P, P], F32, tag="ybn")
                                nc.scalar.activation(
                                    ybn[:, :], pst[:, :], Act.Identity,
                                    bias=bv_sb[layer][:, fc:fc + 1],
                                    scale=gv_sb[layer][:, fc:fc + 1])
                                ey = spool.tile([P, P], F32, tag="ey")
                                nc.scalar.activation(ey[:, :], ybn[:, :],
                                                     Act.Exp)
                                nc.vector.tensor_scalar(
                                    out=ey[:, :], in0=ey[:, :], scalar1=1.0,
                                    scalar2=0.0, op0=AluOp.subtract,
                                    op1=AluOp.min)
                                nc.vector.tensor_scalar(
                                    out=ybn[:, :], in0=ybn[:, :], scalar1=0.0,
                                    scalar2=None, op0=AluOp.max)
                                if layer == 0:
                                    nc.vector.tensor_tensor(
                                        out=xt_sb[1][:, fc, t * P:(t + 1) * P],
                                        in0=ey[:, :], in1=ybn[:, :],
                                        op=AluOp.add)
                                else:
                                    nc.vector.tensor_tensor(
                                        out=cls_in[:, fc, :],
                                        in0=ey[:, :], in1=ybn[:, :],
                                        op=AluOp.add)
                            if layer == 1:
                                # fused classifier for this tile
                                cps = pspool.tile([P, 512], F32, tag="mm",
                                                  name="cps")
                                for kc in range(FC):
                                    nc.tensor.matmul(
                                        cps[:, 0:OUT],
                                        lhsT=cls_in[:, kc, :],
                                        rhs=wct_sb[:, kc, :],
                                        start=(kc == 0), stop=(kc == FC - 1),
                                    )
                                ob = spool.tile([P, OUT], F32, tag="ob")
                                nc.vector.tensor_tensor(
                                    out=ob[:, :], in0=cps[:, 0:OUT],
                                    in1=bc_sb[:, :], op=AluOp.add)
                                nc.sync.dma_start(
                                    out=out_ext[t * P:(t + 1) * P, :],
                                    in_=ob[:, :])
                        offs += S

                if ABL == 1:
                    # classifier over (garbage) layer-1 buffer, timing only
                    for t in range(NT):
                        cps = pspool.tile([P, 512], F32, tag="mm", name="cps1")
                        for kc in range(FC):
                            nc.tensor.matmul(
                                cps[:, 0:OUT],
                                lhsT=xt_sb[1][:, kc, t * P:(t + 1) * P],
                                rhs=wct_sb[:, kc, :],
                                start=(kc == 0), stop=(kc == FC - 1),
                            )
                        ob = spool.tile([P, OUT], F32, tag="ob")
                        nc.vector.tensor_tensor(out=ob[:, :], in0=cps[:, 0:OUT],
                                                in1=bc_sb[:, :], op=AluOp.add)
                        nc.sync.dma_start(out=out_ext[t * P:(t + 1) * P, :],
                                          in_=ob[:, :])

    nc.compile()
    return nc


# --------------------------------------------------------------------------
# entry point
# --------------------------------------------------------------------------

def kernel(x, edge_index, W1, a_src1, a_dst1, bn1_gamma, bn1_beta, bn1_mean,
           bn1_var, W2, a_src2, a_dst2, bn2_gamma, bn2_beta, bn2_mean, bn2_var,
           Wc, bc, _cfg=None, _run_kwargs=None, _bench=0):
    cfg = dict(CFG)
    if _cfg:
        cfg.update(_cfg)
    N, F, OUT = cfg["N"], cfg["F"], cfg["OUT"]
    FC = F // P

    if cfg.get("PERM", 1):
        pos = balance_perm(cfg, edge_index)
        edge_index = pos[np.asarray(edge_index, np.int64)]
    else:
        pos = None
    groups, KA, KB, TOT, per_core, NB, NT = prep_edges(cfg, edge_index)
    nc = build_kernel(cfg, groups, KA, KB, TOT, NB, NT)

    wfull1 = _wfull(W1, a_src1, a_dst1)
    wfull2 = _wfull(W2, a_src2, a_dst2)
    wct = np.ascontiguousarray(np.asarray(Wc, np.float64).T).astype(
        ml_dtypes.bfloat16)
    g1, b1 = _bn_consts(bn1_gamma, bn1_beta, bn1_mean, bn1_var)
    g2, b2 = _bn_consts(bn2_gamma, bn2_beta, bn2_mean, bn2_var)
    bc_rep = np.tile(np.asarray(bc, np.float32)[None, :], (P, 1))

    xpad = np.zeros((NB * NCORES, F), np.float32)
    if pos is not None:
        xpad[pos[:N]] = np.asarray(x, np.float32)
    else:
        xpad[:N] = np.asarray(x, np.float32)
    xt = np.ascontiguousarray(xpad.T).astype(ml_dtypes.bfloat16)  # [F, NPAD]

    in_maps = []
    for k in range(NCORES):
        xk = xt[:, k * NB:(k + 1) * NB].reshape(FC, P, NB)
        in_maps.append(dict(
            xt=np.ascontiguousarray(xk),
            wfull1=np.ascontiguousarray(wfull1.reshape(FC, P, 268)),
            wfull2=np.ascontiguousarray(wfull2.reshape(FC, P, 268)),
            wct=np.ascontiguousarray(wct.reshape(FC, P, OUT)),
            gvec1=g1, bvec1=b1, gvec2=g2, bvec2=b2, bc_rep=bc_rep,
            idxa=per_core[k]["idxa"], idxb=per_core[k]["idxb"],
            dstrel=per_core[k]["dstrel"], dstrelr=per_core[k]["dstrelr"],
            asrcr1=_asrc_rep(a_src1), asrcr2=_asrc_rep(a_src2),
            ident=np.eye(P, dtype=np.float32),
            iotarow=np.tile(np.arange(P, dtype=np.float32)[None, :],
                            (P, 1)).astype(ml_dtypes.bfloat16),
            iotacol=np.ascontiguousarray(
                np.tile(np.arange(P, dtype=np.float32)[:, None], (1, P))),
            ones1=np.ones((1, P), np.float32).astype(ml_dtypes.bfloat16),
        ))

    res = run_bass_kernel_spmd(nc, in_maps, list(range(NCORES)),
                               **(_run_kwargs or {}))
    out = np.concatenate([res.results[k]["out"] for k in range(NCORES)], axis=0)
    if pos is not None:
        out = out[pos[:N]]
    out = out[:N].astype(np.float32)
    if _bench:
        ns = _bench_pjrt(nc, in_maps, _bench)
        return out, ns
    if _run_kwargs is not None:
        return out, res
    return out


def _bench_pjrt(nc, in_maps, iters):
    """Median per-iteration wall time (ns) of the NEFF execution via PJRT,
    device-resident inputs, back-to-back async dispatch."""
    import time
    import jax
    import jax.numpy as jnp
    from jax.sharding import Mesh, PartitionSpec
    from jax.experimental.shard_map import shard_map
    from concourse import bass2jax
    from concourse.bass2jax import _bass_exec_p, partition_id_tensor
    import concourse.mybir as mybir

    n_cores = len(in_maps)
    partition_name = nc.partition_id_tensor.name if nc.partition_id_tensor else None
    in_names, out_names, out_avals, zero_outs = [], [], [], []
    for alloc in nc.m.functions[0].allocations:
        if not isinstance(alloc, mybir.MemoryLocationSet):
            continue
        name = alloc.memorylocations[0].name
        if alloc.kind == "ExternalInput":
            if name != partition_name:
                in_names.append(name)
        elif alloc.kind == "ExternalOutput":
            shape = list(alloc.tensor_shape)
            dt = mybir.dt.np(alloc.dtype)
            out_avals.append(jax.core.ShapedArray(shape, dt))
            out_names.append(name)
            zero_outs.append(np.zeros(shape, dt))
    n_params = len(in_names)
    n_outs = len(out_names)
    in_names.extend(out_names)
    if partition_name is not None:
        in_names.append(partition_name)

    def _body(*args):
        operands = list(args)
        if partition_name is not None:
            operands.append(partition_id_tensor())
        return tuple(_bass_exec_p.bind(
            *operands, out_avals=tuple(out_avals), in_names=tuple(in_names),
            out_names=tuple(out_names), lowering_input_output_aliases=(),
            sim_require_finite=True, sim_require_nnan=True, nc=nc))

    devices = jax.devices()[:n_cores]
    mesh = Mesh(np.asarray(devices), ("core",))
    sharded = jax.jit(
        shard_map(_body, mesh=mesh,
                  in_specs=(PartitionSpec("core"),) * (n_params + n_outs),
                  out_specs=(PartitionSpec("core"),) * n_outs,
                  check_rep=False),
        donate_argnums=(), keep_unused=True)
    per_core = [[np.asarray(m[name]) for name in in_names[:n_params]]
                for m in in_maps]
    concat_in = [np.concatenate([per_core[c][i] for c in range(n_cores)], axis=0)
                 for i in range(n_params)]
    from jax.sharding import NamedSharding
    sh = NamedSharding(mesh, PartitionSpec("core"))
    dev_in = [jax.device_put(a, sh) for a in concat_in]
    zshapes = [(n_cores * z.shape[0], *z.shape[1:]) for z in zero_outs]
    zdtypes = [z.dtype for z in zero_outs]

    dev_zeros = [jax.device_put(np.zeros(s_, d_), sh)
                 for s_, d_ in zip(zshapes, zdtypes)]

    def one_iter():
        return sharded(*dev_in, *dev_zeros)

    jax.block_until_ready(one_iter())
    times = []
    for _ in range(5):
        t0 = time.perf_counter()
        outs = [one_iter() for _ in range(iters)]
        jax.block_until_ready(outs[-1])
        times.append((time.perf_counter() - t0) / iters * 1e9)
    return min(times)


# revision 21
# speedup vs baseline: 1.1884x; 1.1884x over previous
"""GAT (2-layer, 4-head) Trainium2 kernel over 8 NeuronCores — v2.

Strategy:
  * Edges sorted by dst, dst-range partitioned across the 8 cores (each core
    owns N/8 node rows and fully computes their output -> no output
    all-reduce, softmax stats stay core-local).
  * Per layer: node GEMM is data-parallel over the owned node range, the
    augmented node table (h | ones | s_src | s_dst) is AllGathered to every
    core's HBM, then the edge phase gathers h[src] rows with dma_gather and
    performs the segment softmax + weighted scatter-add as one-hot matmuls
    accumulated in PSUM.
  * v2: the per-edge s_dst values come from a transposed one-hot matmul
    against the tile's own s_dst vector (built via a rank-1 PE broadcast of
    the host-known dstrel row + DVE is_equal) instead of a second dma_gather;
    per-(tile,half) slot counts are exact (max over cores) instead of a
    global max; gathers round-robin over 4 SWDGE queues; the classifier is
    fused into layer 2's edge phase.
  * int16 gather indices cap at 32767, so the node table is addressed as two
    halves (rows < VH and the rest) with per-tile A/B edge slot groups.
"""

import sys

if "/opt/trn_rl_repo" not in sys.path:
    sys.path.insert(0, "/opt/trn_rl_repo")

import ml_dtypes
import numpy as np

import concourse.bacc as bacc
import concourse.bass as bass
import concourse.mybir as mybir
import concourse.tile as tile
from concourse.bass_utils import run_bass_kernel_spmd

BF16 = mybir.dt.bfloat16
F32 = mybir.dt.float32
I16 = mybir.dt.int16

NCORES = 8
P = 128

CFG = dict(
    N=50000,
    E=500000,
    F=256,      # feature width (in = hid = 256)
    H=4,
    DH=64,
    OUT=64,
    ROW=384,    # padded bf16 row length of node table (768B, %256B)
    SLOTMAX=28,  # max 128-edge slots per gather group
    NQ=4,       # SWDGE queues for gathers
)


# --------------------------------------------------------------------------
# host-side preparation
# --------------------------------------------------------------------------

def _head_matrix(a):
    """[H, DH] -> block diagonal [F, H] so that s = h @ A."""
    H, DH = np.asarray(a).shape
    A = np.zeros((H * DH, H), np.float64)
    for h in range(H):
        A[h * DH:(h + 1) * DH, h] = np.asarray(a, np.float64)[h]
    return A


def _wfull(W, a_src, a_dst):
    """[W^T | zeros | W^T@Asrc | W^T@Adst] as [F, 268] bf16."""
    W = np.asarray(W, np.float64)
    F = W.shape[1]
    Wt = W.T
    Bs = Wt @ _head_matrix(a_src)
    Bd = Wt @ _head_matrix(a_dst)
    out = np.zeros((F, 268), np.float64)
    out[:, :W.shape[0]] = Wt
    out[:, 260:264] = Bs
    out[:, 264:268] = Bd
    return out.astype(ml_dtypes.bfloat16)


def _bn_consts(gamma, beta, mean, var, eps=1e-5):
    gamma = np.asarray(gamma, np.float64)
    beta = np.asarray(beta, np.float64)
    mean = np.asarray(mean, np.float64)
    var = np.asarray(var, np.float64)
    g = gamma / np.sqrt(var + eps)
    b = beta - mean * g
    F = gamma.shape[0]
    return (
        np.ascontiguousarray(g.reshape(F // P, P).T.astype(np.float32)),
        np.ascontiguousarray(b.reshape(F // P, P).T.astype(np.float32)),
    )


def _wrap_idx(flat):
    """int16 position array -> dma_gather wrapped layout [128, len//16]."""
    n = len(flat)
    assert n % 16 == 0
    w = np.zeros((P, n // 16), np.int16)
    w[:16, :] = np.asarray(flat, np.int16).reshape(-1, 16).T
    w[16:, :] = np.tile(w[:16, :], (7, 1))
    return w


def balance_perm(cfg, edge_index):
    """Permute nodes within each core's range so per-tile (A,B) in-degree
    sums are balanced -> fewer 128-edge gather slots. Returns pos[id]."""
    N = cfg["N"]
    NB = ((N + NCORES - 1) // NCORES + P - 1) // P * P
    NT = NB // P
    RA = (NT // 2) * P
    NPAD = NB * NCORES
    src = np.asarray(edge_index[0], np.int64)
    dst = np.asarray(edge_index[1], np.int64)
    TSPLIT = NT // 2
    # nodes never change half (A = local < RA), so the (inA, inB) in-degree
    # labels stay exact and one packing pass is sufficient
    a_lab = (src % NB) < RA
    inA = np.bincount(dst[a_lab], minlength=NPAD).astype(np.int64)
    inB = np.bincount(dst[~a_lab], minlength=NPAD).astype(np.int64)
    base = 5 * P
    # fixed overflow-tile indices shared by all cores (per-tile slot count
    # is a max over cores): per half-group, A overflow at its low tiles,
    # B overflow at its high tiles
    capA_t = np.full(NT, base, np.int64)
    capB_t = np.full(NT, base, np.int64)
    for t0, nbin in ((0, TSPLIT), (TSPLIT, NT - TSPLIT)):
        needA = needB = 0
        for k in range(NCORES):
            ids = np.arange(k * NB + t0 * P, k * NB + (t0 + nbin) * P)
            needA = max(needA, (inA[ids].sum() - nbin * base + P - 1) // P)
            needB = max(needB, (inB[ids].sum() - nbin * base + P - 1) // P)
        nExA = int(max(needA, 0)) + 2
        nExB = int(max(needB, 0)) + 2
        capA_t[t0:t0 + nExA] += P
        capB_t[t0 + nbin - nExB:t0 + nbin] += P
    pos = np.empty(NPAD, np.int64)
    for k in range(NCORES):
        for t0, nbin in ((0, TSPLIT), (TSPLIT, NT - TSPLIT)):
            lo = k * NB + t0 * P
            ids = np.arange(lo, lo + nbin * P)
            capA = capA_t[t0:t0 + nbin]
            capB = capB_t[t0:t0 + nbin]
            tot = inA[ids] + inB[ids]
            order = np.argsort(-tot, kind="stable")
            sumA = np.zeros(nbin)
            sumB = np.zeros(nbin)
            cnt = np.zeros(nbin)
            lane = np.zeros(nbin, np.int64)
            fA = capA.astype(np.float64)
            fB = capB.astype(np.float64)
            for j in order:
                a, b = inA[ids[j]], inB[ids[j]]
                load = np.maximum((sumA + a) / fA, (sumB + b) / fB)
                load = np.maximum(load, (cnt + 1) / P) + (cnt >= P) * 1e9
                t = int(np.argmin(load))
                sumA[t] += a
                sumB[t] += b
                cnt[t] += 1
                pos[ids[j]] = lo + t * P + lane[t]
                lane[t] += 1
    return pos


def prep_edges(cfg, edge_index):
    """Sort/partition edges; exact per-(tile,half) slot counts (max over
    cores), greedy tile groups bounded by SLOTMAX slots."""
    N, SLOTMAX = cfg["N"], cfg["SLOTMAX"]
    NB = ((N + NCORES - 1) // NCORES + P - 1) // P * P
    NT = NB // P
    SPLIT2 = cfg.get("SPLIT2", 1)
    src = np.asarray(edge_index[0], np.int64)
    dst = np.asarray(edge_index[1], np.int64)
    core = dst // NB
    tilein = (dst % NB) // P
    if SPLIT2:
        # split the shared table at tile TSPLIT: A = local rows < RA, B =
        # rest; both halves must index within int16 across all cores
        TSPLIT = NT // 2
        RA = TSPLIT * P
        RB = NB - RA
        assert NCORES * RA <= 32768 and NCORES * RB <= 32768
        cfg["TSPLIT"] = TSPLIT
        half = ((src % NB) >= RA).astype(np.int64)
    else:
        VH = min(32768, NB * NCORES // 2)
        cfg["VH"] = VH
        cfg["TSPLIT"] = NT
        half = (src >= VH).astype(np.int64)
    order = np.lexsort((src, half, tilein, core))
    sc, tc, hc = core[order], tilein[order], half[order]
    ss, ds = src[order], dst[order]
    if SPLIT2:
        # remap src to half-table indices
        s_core = ss // NB
        s_loc = ss % NB
        ss = np.where(s_loc < RA, s_core * RA + s_loc,
                      s_core * RB + (s_loc - RA))
    else:
        ss = np.where(ss < VH, ss, ss - VH)
    key = (sc * NT + tc) * 2 + hc
    bounds = np.searchsorted(key, np.arange(NCORES * NT * 2 + 1))
    lists = {}
    for k in range(NCORES):
        for t in range(NT):
            for h in (0, 1):
                j = (k * NT + t) * 2 + h
                i0, i1 = bounds[j], bounds[j + 1]
                lists[(k, t, h)] = (ss[i0:i1], ds[i0:i1] % P)
    # exact slot counts per (tile, half): max over cores
    KA = [max(max((len(lists[(k, t, 0)][0]) for k in range(NCORES))), 1)
          for t in range(NT)]
    KB = [max(max((len(lists[(k, t, 1)][0]) for k in range(NCORES))), 1)
          for t in range(NT)]
    KA = [(c + P - 1) // P for c in KA]
    KB = [(c + P - 1) // P for c in KB]

    # greedy groups of consecutive tiles, <= SLOTMAX slots each
    groups = []  # list of (t0, gn, SA, SB)
    t0 = 0
    while t0 < NT:
        gn = 1
        sa, sb = KA[t0], KB[t0]
        while (t0 + gn < NT
               and sa + sb + KA[t0 + gn] + KB[t0 + gn] <= SLOTMAX):
            sa += KA[t0 + gn]
            sb += KB[t0 + gn]
            gn += 1
        groups.append((t0, gn, sa, sb))
        t0 += gn
    TOT = sum(sa + sb for _, _, sa, sb in groups)

    per_core = []
    for k in range(NCORES):
        idxa_cols, idxb_cols, drr_cols = [], [], []
        dstrel = np.full((P, TOT), 128.0, np.float32)
        soff = 0
        for (g0, gn, sa, sb) in groups:
            fa = np.zeros(sa * P, np.int16)
            fb = np.zeros(sb * P, np.int16)
            drow = np.full((sa + sb) * P, 128.0, np.float32)
            aoff = 0
            boff = sa
            for tl in range(gn):
                t = g0 + tl
                for h in (0, 1):
                    s_arr, r_arr = lists[(k, t, h)]
                    n = len(s_arr)
                    if h == 0:
                        base = aoff * P
                        fa[base:base + n] = s_arr.astype(np.int16)
                        slot0 = aoff
                        aoff += KA[t]
                    else:
                        base = (boff - sa) * P
                        fb[base:base + n] = s_arr.astype(np.int16)
                        slot0 = boff
                        boff += KB[t]
                    for i in range(n):
                        dstrel[i % P, soff + slot0 + i // P] = r_arr[i]
                        drow[(slot0 + i // P) * P + i % P] = r_arr[i]
            idxa_cols.append(_wrap_idx(fa))
            idxb_cols.append(_wrap_idx(fb))
            drr_cols.append(drow)
            soff += sa + sb
        per_core.append(dict(
            idxa=np.concatenate(idxa_cols, axis=1),
            idxb=np.concatenate(idxb_cols, axis=1),
            dstrel=dstrel.astype(ml_dtypes.bfloat16),
            dstrelr=np.concatenate(drr_cols)[None, :].astype(ml_dtypes.bfloat16),
        ))
    return groups, KA, KB, TOT, per_core, NB, NT


# --------------------------------------------------------------------------
# device kernel
# --------------------------------------------------------------------------

def apx(base_ap, pairs, extra_offset=0):
    return bass.AP(base_ap.tensor, base_ap.offset + extra_offset,
                   [list(p) for p in pairs])


def build_kernel(cfg, groups, KA, KB, TOT, NB, NT):
    F, H, DH, OUT = cfg["F"], cfg["H"], cfg["DH"], cfg["OUT"]
    ROW, TSPLIT = cfg["ROW"], cfg["TSPLIT"]
    SLOTMAX, NQ = cfg["SLOTMAX"], cfg["NQ"]
    SPLIT2 = cfg.get("SPLIT2", 1)
    FC = F // P
    NPAD = NB * NCORES
    RA = TSPLIT * P
    RB = NB - RA
    AluOp = mybir.AluOpType
    Act = mybir.ActivationFunctionType
    ABL = cfg.get("ABL", 5)

    nc = bacc.Bacc("TRN2", target_bir_lowering=False, debug=False,
                   num_devices=NCORES, num_swdge_queues=NQ)

    # ---- I/O ----
    xt_in = nc.declare_dram_parameter("xt", [FC, P, NB], BF16, isOutput=False)
    wf_in = [nc.declare_dram_parameter(f"wfull{l + 1}", [FC, P, 268], BF16,
                                       isOutput=False) for l in range(2)]
    wct_in = nc.declare_dram_parameter("wct", [FC, P, OUT], BF16, isOutput=False)
    gv_in = [nc.declare_dram_parameter(f"gvec{l + 1}", [P, FC], F32,
                                       isOutput=False) for l in range(2)]
    bv_in = [nc.declare_dram_parameter(f"bvec{l + 1}", [P, FC], F32,
                                       isOutput=False) for l in range(2)]
    bc_in = nc.declare_dram_parameter("bc_rep", [P, OUT], F32, isOutput=False)
    SA_tot = sum(sa for _, _, sa, _ in groups)
    SB_tot = sum(sb for _, _, _, sb in groups)
    idxa_in = nc.declare_dram_parameter("idxa", [P, SA_tot * 8], I16,
                                        isOutput=False)
    idxb_in = nc.declare_dram_parameter("idxb", [P, SB_tot * 8], I16,
                                        isOutput=False)
    ident_in = nc.declare_dram_parameter("ident", [P, P], F32, isOutput=False)
    iota_in = nc.declare_dram_parameter("iotarow", [P, P], BF16, isOutput=False)
    iotac_in = nc.declare_dram_parameter("iotacol", [P, P], F32, isOutput=False)
    ones1_in = nc.declare_dram_parameter("ones1", [1, P], BF16, isOutput=False)
    dstrel_in = nc.declare_dram_parameter("dstrel", [P, TOT], BF16,
                                          isOutput=False)
    drr_in = nc.declare_dram_parameter("dstrelr", [1, TOT * P], BF16,
                                       isOutput=False)
    out_ext = nc.declare_dram_parameter("out", [NB, OUT], F32, isOutput=True)

    if SPLIT2:
        haug_ownA = [nc.dram_tensor(f"haug_ownA{l}", [RA, ROW], BF16)
                     for l in (0, 1)]
        haug_ownB = [nc.dram_tensor(f"haug_ownB{l}", [RB, ROW], BF16)
                     for l in (0, 1)]
        haug_allA = [nc.dram_tensor(f"haug_allA{l}", [NCORES, RA, ROW], BF16,
                                    addr_space="Shared") for l in (0, 1)]
        haug_allB = [nc.dram_tensor(f"haug_allB{l}", [NCORES, RB, ROW], BF16,
                                    addr_space="Shared") for l in (0, 1)]
    else:
        haug_own = [nc.dram_tensor(f"haug_own{l}", [NB, ROW], BF16)
                    for l in (0, 1)]
        haug_all = [nc.dram_tensor(f"haug_all{l}", [NCORES, NB, ROW], BF16,
                                   addr_space="Shared") for l in (0, 1)]


    with tile.TileContext(nc) as tc:
        with (
            tc.tile_pool(name="const", bufs=1) as cpool,
            tc.tile_pool(name="persist", bufs=1) as ppool,
            tc.tile_pool(name="work", bufs=2) as wpool,
            tc.tile_pool(name="works", bufs=4) as spool,
            tc.tile_pool(name="gath", bufs=2) as gpool,
            tc.tile_pool(name="psmm", bufs=2, space="PSUM") as pspool,
            tc.tile_pool(name="psacc", bufs=2, space="PSUM") as accpool,
            tc.tile_pool(name="pssd", bufs=2, space="PSUM") as sdpool,
        ):
            # ---- constants ----
            ident = cpool.tile([P, P], F32)
            nc.sync.dma_start(out=ident[:, :], in_=ident_in[:, :])
            iota_bf = cpool.tile([P, P], BF16)
            nc.sync.dma_start(out=iota_bf[:, :], in_=iota_in[:, :])
            iotac = cpool.tile([P, P], F32)
            nc.sync.dma_start(out=iotac[:, :], in_=iotac_in[:, :])
            ones1 = cpool.tile([1, P], BF16)
            nc.sync.dma_start(out=ones1[:, :], in_=ones1_in[:, :])
            wf_sb = [cpool.tile([P, FC, 268], BF16, tag=f"wf{l}", name=f"wf{l}")
                     for l in range(2)]
            for l in range(2):
                nc.sync.dma_start(out=wf_sb[l][:, :, :],
                                  in_=wf_in[l].rearrange("c p n -> p c n"))
            wct_sb = cpool.tile([P, FC, OUT], BF16)
            nc.sync.dma_start(out=wct_sb[:, :, :],
                              in_=wct_in.rearrange("c p n -> p c n"))
            gv_sb = [cpool.tile([P, FC], F32, tag=f"gv{l}", name=f"gv{l}")
                     for l in range(2)]
            bv_sb = [cpool.tile([P, FC], F32, tag=f"bv{l}", name=f"bv{l}")
                     for l in range(2)]
            for l in range(2):
                nc.sync.dma_start(out=gv_sb[l][:, :], in_=gv_in[l][:, :])
                nc.sync.dma_start(out=bv_sb[l][:, :], in_=bv_in[l][:, :])
            bc_sb = cpool.tile([P, OUT], F32)
            nc.sync.dma_start(out=bc_sb[:, :], in_=bc_in[:, :])
            dstrel_sb = cpool.tile([P, TOT], BF16)
            nc.sync.dma_start(out=dstrel_sb[:, :], in_=dstrel_in[:, :])
            SA_tot_ = sum(sa for _, _, sa, _ in groups)
            SB_tot_ = sum(sb for _, _, _, sb in groups)
            idxa_sb = cpool.tile([P, SA_tot_ * 8], I16)
            nc.sync.dma_start(out=idxa_sb[:, :], in_=idxa_in[:, :])
            idxb_sb = cpool.tile([P, SB_tot_ * 8], I16)
            nc.sync.dma_start(out=idxb_sb[:, :], in_=idxb_in[:, :])

            # buf0 = x (layer-1 input, never overwritten)
            # buf1 = layer-1 edge output (layer-2 input)
            xt_sb = [ppool.tile([P, FC, NB], BF16, tag=f"xt{l}", name=f"xt{l}")
                     for l in range(2)]
            nc.sync.dma_start(out=xt_sb[0][:, :, :],
                              in_=xt_in.rearrange("c p n -> p c n"))
            sdst_sb = ppool.tile([P, NT * H], BF16)

            for rep_ in range(cfg.get("REPEAT", 1)):
                for layer in (0, 1):
                    wfl = wf_sb[layer]
                    xt = xt_sb[layer]

                    # ---- node GEMM -> haug_own halves + local s_dst ----
                    for t in range(NT):
                        ps = pspool.tile([P, 512], F32, tag="mm")
                        for kc in range(FC):
                            nc.tensor.matmul(
                                ps[:, 0:268],
                                lhsT=xt[:, kc, t * P:(t + 1) * P],
                                rhs=wfl[:, kc, :],
                                start=(kc == 0), stop=(kc == FC - 1),
                            )
                        stg = spool.tile([P, ROW], BF16, tag="gemmout")
                        nc.scalar.copy(stg[:, 0:268], ps[:, 0:268])
                        nc.vector.memset(stg[:, 268:ROW], 0.0)
                        if not SPLIT2:
                            nc.sync.dma_start(
                                out=haug_own[layer][t * P:(t + 1) * P, :],
                                in_=stg[:, :])
                        elif t < TSPLIT:
                            nc.sync.dma_start(
                                out=haug_ownA[layer][t * P:(t + 1) * P, :],
                                in_=stg[:, :])
                        else:
                            t2 = t - TSPLIT
                            nc.sync.dma_start(
                                out=haug_ownB[layer][t2 * P:(t2 + 1) * P, :],
                                in_=stg[:, :])
                        nc.vector.tensor_copy(sdst_sb[:, t * H:(t + 1) * H],
                                              ps[:, 264:268])
                        if SPLIT2 and t == TSPLIT - 1:
                            # ---- share first half of the node table ----
                            if ABL == 4:
                                nc.sync.dma_start(
                                    out=haug_allA[layer][0, :, :],
                                    in_=haug_ownA[layer][:, :])
                            else:
                                nc.gpsimd.collective_compute(
                                    "AllGather", AluOp.bypass,
                                    replica_groups=[list(range(NCORES))],
                                    ins=[haug_ownA[layer][:, :]],
                                    outs=[haug_allA[layer][:, :, :]],
                                )
                    # ---- share second half (or, SPLIT2=0, everything) ----
                    if SPLIT2:
                        c_in, c_out = haug_ownB[layer], haug_allB[layer]
                    else:
                        c_in, c_out = haug_own[layer], haug_all[layer]
                    if ABL == 4:
                        nc.sync.dma_start(out=c_out[0, :, :], in_=c_in[:, :])
                    else:
                        nc.gpsimd.collective_compute(
                            "AllGather", AluOp.bypass,
                            replica_groups=[list(range(NCORES))],
                            ins=[c_in[:, :]],
                            outs=[c_out[:, :, :]],
                        )
                    if SPLIT2:
                        hflatA = haug_allA[layer].rearrange("c n d -> (c n) d")
                        hflatB = haug_allB[layer].rearrange("c n d -> (c n) d")
                    else:
                        hfl = haug_all[layer].rearrange("c n d -> (c n) d")
                        VH = cfg["VH"]
                        hflatA = hfl[0:VH, :]
                        hflatB = hfl[VH:NPAD, :]
                    if ABL == 1:
                        if layer == 0:
                            nc.vector.memset(xt_sb[1][:, :, :], 0.1)
                        continue

                    # ---- edge phase ----
                    offa = offb = offs = 0
                    for gi, (g0, gn, sa, sb) in enumerate(groups):
                        S = sa + sb
                        drr_t = wpool.tile([1, SLOTMAX * P], BF16, tag="drr")
                        nc.sync.dma_start(
                            out=drr_t[:, 0:S * P],
                            in_=drr_in[:, offs * P:(offs + S) * P])

                        gat = gpool.tile([P, SLOTMAX, ROW], BF16, tag="gat")
                        if ABL == 2:
                            nc.vector.memset(gat[:, :, :], 0.05)
                            nc.vector.tensor_copy(gat[:, 0:1, 0:8],
                                                  idxa_sb[:, 0:8])
                        else:
                            nc.gpsimd.dma_gather(
                                out_ap=gat[:, 0:sa, :], in_ap=hflatA[:, :],
                                idxs_ap=idxa_sb[:, offa:offa + sa * 8],
                                num_idxs=sa * P,
                                num_idxs_reg=sa * P, elem_size=ROW,
                                single_packet=False,
                                queue_num=(2 * gi) % NQ)
                            nc.gpsimd.dma_gather(
                                out_ap=gat[:, sa:S, :], in_ap=hflatB[:, :],
                                idxs_ap=idxb_sb[:, offb:offb + sb * 8],
                                num_idxs=sb * P,
                                num_idxs_reg=sb * P, elem_size=ROW,
                                single_packet=False,
                                queue_num=(2 * gi + 1) % NQ)
                        offa += sa * 8
                        offb += sb * 8

                        # transposed one-hot: ohT[n, s, e] = (n == dstrel[s,e])
                        ohT = wpool.tile([P, SLOTMAX, P], BF16, tag="ohT")
                        nch = (S + 3) // 4 if ABL != 10 else 0
                        for c in range(nch):
                            cols = min(4, S - 4 * c) * P
                            bc_ps = pspool.tile([P, 512], F32, tag="mm",
                                                name="bc_ps")
                            nc.tensor.matmul(
                                bc_ps[:, 0:cols],
                                lhsT=ones1[:, :],
                                rhs=drr_t[:, 4 * c * P:4 * c * P + cols],
                                start=True, stop=True)
                            iotac_ap = iotac[:, :]
                            nc.vector.tensor_tensor(
                                out=ohT[:, 4 * c:4 * c + cols // P, :],
                                in0=apx(iotac_ap,
                                        [iotac_ap.ap[0], [0, cols // P],
                                         [1, P]]),
                                in1=apx(bc_ps[:, :],
                                        [bc_ps[:, :].ap[0], [P, cols // P],
                                         [1, P]]),
                                op=AluOp.is_equal)

                        # per-edge s_dst via ohT matmul against own s_dst
                        sdacc = sdpool.tile([P, SLOTMAX, H], F32, tag="sdacc")
                        if ABL != 10:
                            sl = 0
                            for h_ in (0, 1):
                                for tl in range(gn):
                                    t = g0 + tl
                                    for _ in range(KA[t] if h_ == 0 else KB[t]):
                                        nc.tensor.matmul(
                                            sdacc[:, sl, :],
                                            lhsT=ohT[:, sl, :],
                                            rhs=sdst_sb[:, t * H:(t + 1) * H],
                                            start=True, stop=True)
                                        sl += 1

                        # e = lrelu(ssrc + sdst); w = exp(e)
                        ef = spool.tile([P, SLOTMAX, H], F32, tag="ef")
                        if ABL == 10:
                            nc.vector.tensor_scalar(
                                out=ef[:, 0:S, :], in0=gat[:, 0:S, 260:264],
                                scalar1=1.0, scalar2=None, op0=AluOp.mult)
                        else:
                            nc.vector.tensor_tensor(
                                out=ef[:, 0:S, :], in0=gat[:, 0:S, 260:264],
                                in1=sdacc[:, 0:S, :], op=AluOp.add)
                        efs = spool.tile([P, SLOTMAX, H], F32, tag="efs")
                        nc.vector.tensor_scalar(
                            out=efs[:, 0:S, :], in0=ef[:, 0:S, :], scalar1=0.2,
                            scalar2=None, op0=AluOp.mult)
                        nc.vector.tensor_tensor(
                            out=ef[:, 0:S, :], in0=ef[:, 0:S, :],
                            in1=efs[:, 0:S, :], op=AluOp.max)
                        wexp = spool.tile([P, SLOTMAX, H], BF16, tag="wexp")
                        nc.scalar.activation(wexp[:, 0:S, :], ef[:, 0:S, :],
                                             Act.Exp)

                        # one-hot [128e, S, 128n]
                        oh = wpool.tile([P, SLOTMAX, P], BF16, tag="oh")
                        dr = dstrel_sb[:, offs:offs + S]
                        iota_ap = iota_bf[:, :]
                        nc.vector.tensor_tensor(
                            out=oh[:, 0:S, :],
                            in0=apx(iota_ap, [iota_ap.ap[0], [0, S], [1, P]]),
                            in1=dr.to_broadcast([P, S, P]),
                            op=AluOp.is_equal)

                        # scale gathered rows by w; cols 256:260 = w itself
                        hts = wpool.tile([P, SLOTMAX, 260], BF16, tag="hts")
                        if ABL == 12:
                            nc.vector.tensor_copy(hts[:, 0:1, :],
                                                  gat[:, 0:1, 0:260])
                        else:
                            nc.vector.tensor_tensor(
                                out=hts[:, 0:S, 0:256].rearrange(
                                    "p s (h d) -> p s h d", h=H),
                                in0=gat[:, 0:S, 0:256].rearrange(
                                    "p s (h d) -> p s h d", h=H),
                                in1=wexp[:, 0:S, :].to_broadcast([P, S, H, DH]),
                                op=AluOp.mult)
                            nc.vector.tensor_copy(hts[:, 0:S, 256:260],
                                                  wexp[:, 0:S, :])

                        # scatter + normalize + BN/ELU per tile
                        aoff = 0
                        boff = sa
                        for tl in range(gn):
                            t = g0 + tl
                            slots = ([aoff + s for s in range(KA[t])] +
                                     [boff + s for s in range(KB[t])])
                            aoff += KA[t]
                            boff += KB[t]
                            if ABL == 11:
                                slots = slots[:1]
                            K_t = len(slots)
                            acc = accpool.tile([P, 260], F32, tag="acc")
                            rsrc = gat if ABL == 12 else hts
                            for j, slx in enumerate(slots):
                                nc.tensor.matmul(
                                    acc[:, :],
                                    lhsT=oh[:, slx, :],
                                    rhs=rsrc[:, slx, 0:260],
                                    start=(j == 0), stop=(j == K_t - 1),
                                )
                            tmax = spool.tile([P, H], F32, tag="tmax")
                            nc.vector.tensor_scalar(
                                out=tmax[:, :], in0=acc[:, 256:260],
                                scalar1=1e-9, scalar2=None, op0=AluOp.max)
                            rec = spool.tile([P, H], F32, tag="rec")
                            nc.vector.reciprocal(rec[:, :], tmax[:, :])
                            zsb = spool.tile([P, F], F32, tag="zsb")
                            nc.vector.tensor_tensor(
                                out=zsb[:, :].rearrange("p (h d) -> p h d",
                                                        h=H),
                                in0=acc[:, 0:256].rearrange("p (h d) -> p h d",
                                                            h=H),
                                in1=rec[:, :].to_broadcast([P, H, DH]),
                                op=AluOp.mult)
                            # transpose + BN + ELU per feature chunk
                            cls_in = spool.tile([P, FC, P], BF16, tag="clsin")
                            for fc in range(FC):
                                pst = pspool.tile([P, P], F32, tag="ptr")
                                nc.tensor.transpose(
                                    pst[:, :], zsb[:, fc * P:(fc + 1) * P],
                                    ident[:, :])
                                ybn = spool.tile([P, P], F32, tag="ybn")
                                nc.scalar.activation(
                                    ybn[:, :], pst[:, :], Act.Identity,
                                    bias=bv_sb[layer][:, fc:fc + 1],
                                    scale=gv_sb[layer][:, fc:fc + 1])
                                ey = spool.tile([P, P], F32, tag="ey")
                                nc.scalar.activation(ey[:, :], ybn[:, :],
                                                     Act.Exp)
                                nc.vector.tensor_scalar(
                                    out=ey[:, :], in0=ey[:, :], scalar1=1.0,
                                    scalar2=0.0, op0=AluOp.subtract,
                                    op1=AluOp.min)
                                nc.vector.tensor_scalar(
                                    out=ybn[:, :], in0=ybn[:, :], scalar1=0.0,
                                    scalar2=None, op0=AluOp.max)
                                if layer == 0:
                                    nc.vector.tensor_tensor(
                                        out=xt_sb[1][:, fc, t * P:(t + 1) * P],
                                        in0=ey[:, :], in1=ybn[:, :],
                                        op=AluOp.add)
                                else:
                                    nc.vector.tensor_tensor(
                                        out=cls_in[:, fc, :],
                                        in0=ey[:, :], in1=ybn[:, :],
                                        op=AluOp.add)
                            if layer == 1:
                                # fused classifier for this tile
                                cps = pspool.tile([P, 512], F32, tag="mm",
                                                  name="cps")
                                for kc in range(FC):
                                    nc.tensor.matmul(
                                        cps[:, 0:OUT],
                                        lhsT=cls_in[:, kc, :],
                                        rhs=wct_sb[:, kc, :],
                                        start=(kc == 0), stop=(kc == FC - 1),
                                    )
                                ob = spool.tile([P, OUT], F32, tag="ob")
                                nc.vector.tensor_tensor(
                                    out=ob[:, :], in0=cps[:, 0:OUT],
                                    in1=bc_sb[:, :], op=AluOp.add)
                                nc.sync.dma_start(
                                    out=out_ext[t * P:(t + 1) * P, :],
                                    in_=ob[:, :])
                        offs += S

                if ABL == 1:
                    # classifier over (garbage) layer-1 buffer, timing only
                    for t in range(NT):
                        cps = pspool.tile([P, 512], F32, tag="mm", name="cps1")
                        for kc in range(FC):
                            nc.tensor.matmul(
                                cps[:, 0:OUT],
                                lhsT=xt_sb[1][:, kc, t * P:(t + 1) * P],
                                rhs=wct_sb[:, kc, :],
                                start=(kc == 0), stop=(kc == FC - 1),
                            )
                        ob = spool.tile([P, OUT], F32, tag="ob")
                        nc.vector.tensor_tensor(out=ob[:, :], in0=cps[:, 0:OUT],
                                                in1=bc_sb[:, :], op=AluOp.add)
                        nc.sync.dma_start(out=out_ext[t * P:(t + 1) * P, :],
                                          in_=ob[:, :])

    nc.compile()
    return nc


# --------------------------------------------------------------------------
# entry point
# --------------------------------------------------------------------------

def kernel(x, edge_index, W1, a_src1, a_dst1, bn1_gamma, bn1_beta, bn1_mean,
           bn1_var, W2, a_src2, a_dst2, bn2_gamma, bn2_beta, bn2_mean, bn2_var,
           Wc, bc, _cfg=None, _run_kwargs=None, _bench=0):
    cfg = dict(CFG)
    if _cfg:
        cfg.update(_cfg)
    N, F, OUT = cfg["N"], cfg["F"], cfg["OUT"]
    FC = F // P

    if cfg.get("PERM", 1):
        pos = balance_perm(cfg, edge_index)
        edge_index = pos[np.asarray(edge_index, np.int64)]
    else:
        pos = None
    groups, KA, KB, TOT, per_core, NB, NT = prep_edges(cfg, edge_index)
    nc = build_kernel(cfg, groups, KA, KB, TOT, NB, NT)

    wfull1 = _wfull(W1, a_src1, a_dst1)
    wfull2 = _wfull(W2, a_src2, a_dst2)
    wct = np.ascontiguousarray(np.asarray(Wc, np.float64).T).astype(
        ml_dtypes.bfloat16)
    g1, b1 = _bn_consts(bn1_gamma, bn1_beta, bn1_mean, bn1_var)
    g2, b2 = _bn_consts(bn2_gamma, bn2_beta, bn2_mean, bn2_var)
    bc_rep = np.tile(np.asarray(bc, np.float32)[None, :], (P, 1))

    xpad = np.zeros((NB * NCORES, F), np.float32)
    if pos is not None:
        xpad[pos[:N]] = np.asarray(x, np.float32)
    else:
        xpad[:N] = np.asarray(x, np.float32)
    xt = np.ascontiguousarray(xpad.T).astype(ml_dtypes.bfloat16)  # [F, NPAD]

    in_maps = []
    for k in range(NCORES):
        xk = xt[:, k * NB:(k + 1) * NB].reshape(FC, P, NB)
        in_maps.append(dict(
            xt=np.ascontiguousarray(xk),
            wfull1=np.ascontiguousarray(wfull1.reshape(FC, P, 268)),
            wfull2=np.ascontiguousarray(wfull2.reshape(FC, P, 268)),
            wct=np.ascontiguousarray(wct.reshape(FC, P, OUT)),
            gvec1=g1, bvec1=b1, gvec2=g2, bvec2=b2, bc_rep=bc_rep,
            idxa=per_core[k]["idxa"], idxb=per_core[k]["idxb"],
            dstrel=per_core[k]["dstrel"], dstrelr=per_core[k]["dstrelr"],
            ident=np.eye(P, dtype=np.float32),
            iotarow=np.tile(np.arange(P, dtype=np.float32)[None, :],
                            (P, 1)).astype(ml_dtypes.bfloat16),
            iotacol=np.ascontiguousarray(
                np.tile(np.arange(P, dtype=np.float32)[:, None], (1, P))),
            ones1=np.ones((1, P), np.float32).astype(ml_dtypes.bfloat16),
        ))

    res = run_bass_kernel_spmd(nc, in_maps, list(range(NCORES)),
                               **(_run_kwargs or {}))
    out = np.concatenate([res.results[k]["out"] for k in range(NCORES)], axis=0)
    if pos is not None:
        out = out[pos[:N]]
    out = out[:N].astype(np.float32)
    if _bench:
        ns = _bench_pjrt(nc, in_maps, _bench)
        return out, ns
    if _run_kwargs is not None:
        return out, res
    return out


def _bench_pjrt(nc, in_maps, iters):
    """Median per-iteration wall time (ns) of the NEFF execution via PJRT,
    device-resident inputs, back-to-back async dispatch."""
    import time
    import jax
    import jax.numpy as jnp
    from jax.sharding import Mesh, PartitionSpec
    from jax.experimental.shard_map import shard_map
    from concourse import bass2jax
    from concourse.bass2jax import _bass_exec_p, partition_id_tensor
    import concourse.mybir as mybir

    n_cores = len(in_maps)
    partition_name = nc.partition_id_tensor.name if nc.partition_id_tensor else None
    in_names, out_names, out_avals, zero_outs = [], [], [], []
    for alloc in nc.m.functions[0].allocations:
        if not isinstance(alloc, mybir.MemoryLocationSet):
            continue
        name = alloc.memorylocations[0].name
        if alloc.kind == "ExternalInput":
            if name != partition_name:
                in_names.append(name)
        elif alloc.kind == "ExternalOutput":
            shape = list(alloc.tensor_shape)
            dt = mybir.dt.np(alloc.dtype)
            out_avals.append(jax.core.ShapedArray(shape, dt))
            out_names.append(name)
            zero_outs.append(np.zeros(shape, dt))
    n_params = len(in_names)
    n_outs = len(out_names)
    in_names.extend(out_names)
    if partition_name is not None:
        in_names.append(partition_name)

    def _body(*args):
        operands = list(args)
        if partition_name is not None:
            operands.append(partition_id_tensor())
        return tuple(_bass_exec_p.bind(
            *operands, out_avals=tuple(out_avals), in_names=tuple(in_names),
            out_names=tuple(out_names), lowering_input_output_aliases=(),
            sim_require_finite=True, sim_require_nnan=True, nc=nc))

    devices = jax.devices()[:n_cores]
    mesh = Mesh(np.asarray(devices), ("core",))
    sharded = jax.jit(
        shard_map(_body, mesh=mesh,
                  in_specs=(PartitionSpec("core"),) * (n_params + n_outs),
                  out_specs=(PartitionSpec("core"),) * n_outs,
                  check_rep=False),
        donate_argnums=(), keep_unused=True)
    per_core = [[np.asarray(m[name]) for name in in_names[:n_params]]
                for m in in_maps]
    concat_in = [np.concatenate([per_core[c][i] for c in range(n_cores)], axis=0)
                 for i in range(n_params)]
    from jax.sharding import NamedSharding
    sh = NamedSharding(mesh, PartitionSpec("core"))
    dev_in = [jax.device_put(a, sh) for a in concat_in]
    zshapes = [(n_cores * z.shape[0], *z.shape[1:]) for z in zero_outs]
    zdtypes = [z.dtype for z in zero_outs]

    dev_zeros = [jax.device_put(np.zeros(s_, d_), sh)
                 for s_, d_ in zip(zshapes, zdtypes)]

    def one_iter():
        return sharded(*dev_in, *dev_zeros)

    jax.block_until_ready(one_iter())
    times = []
    for _ in range(5):
        t0 = time.perf_counter()
        outs = [one_iter() for _ in range(iters)]
        jax.block_until_ready(outs[-1])
        times.append((time.perf_counter() - t0) / iters * 1e9)
    return min(times)


# revision 22
# speedup vs baseline: 1.5106x; 1.2711x over previous
"""GAT (2-layer, 4-head) Trainium2 kernel over 8 NeuronCores — v2.

Strategy:
  * Edges sorted by dst, dst-range partitioned across the 8 cores (each core
    owns N/8 node rows and fully computes their output -> no output
    all-reduce, softmax stats stay core-local).
  * Per layer: node GEMM is data-parallel over the owned node range, the
    augmented node table (h | ones | s_src | s_dst) is AllGathered to every
    core's HBM, then the edge phase gathers h[src] rows with dma_gather and
    performs the segment softmax + weighted scatter-add as one-hot matmuls
    accumulated in PSUM.
  * v2: the per-edge s_dst values come from a transposed one-hot matmul
    against the tile's own s_dst vector (built via a rank-1 PE broadcast of
    the host-known dstrel row + DVE is_equal) instead of a second dma_gather;
    per-(tile,half) slot counts are exact (max over cores) instead of a
    global max; gathers round-robin over 4 SWDGE queues; the classifier is
    fused into layer 2's edge phase.
  * int16 gather indices cap at 32767, so the node table is addressed as two
    halves (rows < VH and the rest) with per-tile A/B edge slot groups.
"""

import sys

if "/opt/trn_rl_repo" not in sys.path:
    sys.path.insert(0, "/opt/trn_rl_repo")

import ml_dtypes
import numpy as np

import concourse.bacc as bacc
import concourse.bass as bass
import concourse.mybir as mybir
import concourse.tile as tile
from concourse.bass_utils import run_bass_kernel_spmd

BF16 = mybir.dt.bfloat16
F32 = mybir.dt.float32
I16 = mybir.dt.int16

NCORES = 8
P = 128

CFG = dict(
    N=50000,
    E=500000,
    F=256,      # feature width (in = hid = 256)
    H=4,
    DH=64,
    OUT=64,
    ROW=384,    # padded bf16 row length of node table (768B, %256B)
    SLOTMAX=28,  # max 128-edge slots per gather group
    NQ=4,       # SWDGE queues for gathers
)


# --------------------------------------------------------------------------
# host-side preparation
# --------------------------------------------------------------------------

def _head_matrix(a):
    """[H, DH] -> block diagonal [F, H] so that s = h @ A."""
    H, DH = np.asarray(a).shape
    A = np.zeros((H * DH, H), np.float64)
    for h in range(H):
        A[h * DH:(h + 1) * DH, h] = np.asarray(a, np.float64)[h]
    return A


def _wfull(W, a_src, a_dst):
    """[W^T | zeros | W^T@Asrc | W^T@Adst] as [F, 268] bf16."""
    W = np.asarray(W, np.float64)
    F = W.shape[1]
    Wt = W.T
    Bs = Wt @ _head_matrix(a_src)
    Bd = Wt @ _head_matrix(a_dst)
    out = np.zeros((F, 268), np.float64)
    out[:, :W.shape[0]] = Wt
    out[:, 260:264] = Bs
    out[:, 264:268] = Bd
    return out.astype(ml_dtypes.bfloat16)


def _bn_consts(gamma, beta, mean, var, eps=1e-5):
    gamma = np.asarray(gamma, np.float64)
    beta = np.asarray(beta, np.float64)
    mean = np.asarray(mean, np.float64)
    var = np.asarray(var, np.float64)
    g = gamma / np.sqrt(var + eps)
    b = beta - mean * g
    F = gamma.shape[0]
    return (
        np.ascontiguousarray(g.reshape(F // P, P).T.astype(np.float32)),
        np.ascontiguousarray(b.reshape(F // P, P).T.astype(np.float32)),
    )


def _wrap_idx(flat):
    """int16 position array -> dma_gather wrapped layout [128, len//16]."""
    n = len(flat)
    assert n % 16 == 0
    w = np.zeros((P, n // 16), np.int16)
    w[:16, :] = np.asarray(flat, np.int16).reshape(-1, 16).T
    w[16:, :] = np.tile(w[:16, :], (7, 1))
    return w


def balance_perm(cfg, edge_index):
    """Permute nodes within each core's range so per-tile (A,B) in-degree
    sums are balanced -> fewer 128-edge gather slots. Returns pos[id]."""
    N = cfg["N"]
    NB = ((N + NCORES - 1) // NCORES + P - 1) // P * P
    NT = NB // P
    RA = (NT // 2) * P
    NPAD = NB * NCORES
    src = np.asarray(edge_index[0], np.int64)
    dst = np.asarray(edge_index[1], np.int64)
    TSPLIT = NT // 2
    # nodes never change half (A = local < RA), so the (inA, inB) in-degree
    # labels stay exact and one packing pass is sufficient
    a_lab = (src % NB) < RA
    inA = np.bincount(dst[a_lab], minlength=NPAD).astype(np.int64)
    inB = np.bincount(dst[~a_lab], minlength=NPAD).astype(np.int64)
    base = 5 * P
    # fixed overflow-tile indices shared by all cores (per-tile slot count
    # is a max over cores): per half-group, A overflow at its low tiles,
    # B overflow at its high tiles
    capA_t = np.full(NT, base, np.int64)
    capB_t = np.full(NT, base, np.int64)
    for t0, nbin in ((0, TSPLIT), (TSPLIT, NT - TSPLIT)):
        needA = needB = 0
        for k in range(NCORES):
            ids = np.arange(k * NB + t0 * P, k * NB + (t0 + nbin) * P)
            needA = max(needA, (inA[ids].sum() - nbin * base + P - 1) // P)
            needB = max(needB, (inB[ids].sum() - nbin * base + P - 1) // P)
        nExA = int(max(needA, 0)) + 2
        nExB = int(max(needB, 0)) + 2
        capA_t[t0:t0 + nExA] += P
        capB_t[t0 + nbin - nExB:t0 + nbin] += P
    pos = np.empty(NPAD, np.int64)
    for k in range(NCORES):
        for t0, nbin in ((0, TSPLIT), (TSPLIT, NT - TSPLIT)):
            lo = k * NB + t0 * P
            ids = np.arange(lo, lo + nbin * P)
            capA = capA_t[t0:t0 + nbin]
            capB = capB_t[t0:t0 + nbin]
            tot = inA[ids] + inB[ids]
            order = np.argsort(-tot, kind="stable")
            sumA = np.zeros(nbin)
            sumB = np.zeros(nbin)
            cnt = np.zeros(nbin)
            lane = np.zeros(nbin, np.int64)
            fA = capA.astype(np.float64)
            fB = capB.astype(np.float64)
            for j in order:
                a, b = inA[ids[j]], inB[ids[j]]
                load = np.maximum((sumA + a) / fA, (sumB + b) / fB)
                load = np.maximum(load, (cnt + 1) / P) + (cnt >= P) * 1e9
                t = int(np.argmin(load))
                sumA[t] += a
                sumB[t] += b
                cnt[t] += 1
                pos[ids[j]] = lo + t * P + lane[t]
                lane[t] += 1
    return pos


def prep_edges(cfg, edge_index):
    """Sort/partition edges; exact per-(tile,half) slot counts (max over
    cores), greedy tile groups bounded by SLOTMAX slots."""
    N, SLOTMAX = cfg["N"], cfg["SLOTMAX"]
    NB = ((N + NCORES - 1) // NCORES + P - 1) // P * P
    NT = NB // P
    SPLIT2 = cfg.get("SPLIT2", 1)
    src = np.asarray(edge_index[0], np.int64)
    dst = np.asarray(edge_index[1], np.int64)
    core = dst // NB
    tilein = (dst % NB) // P
    if SPLIT2:
        # split the shared table at tile TSPLIT: A = local rows < RA, B =
        # rest; both halves must index within int16 across all cores
        TSPLIT = NT // 2
        RA = TSPLIT * P
        RB = NB - RA
        assert NCORES * RA <= 32768 and NCORES * RB <= 32768
        cfg["TSPLIT"] = TSPLIT
        half = ((src % NB) >= RA).astype(np.int64)
    else:
        VH = min(32768, NB * NCORES // 2)
        cfg["VH"] = VH
        cfg["TSPLIT"] = NT
        half = (src >= VH).astype(np.int64)
    order = np.lexsort((src, half, tilein, core))
    sc, tc, hc = core[order], tilein[order], half[order]
    ss, ds = src[order], dst[order]
    if SPLIT2:
        # remap src to half-table indices
        s_core = ss // NB
        s_loc = ss % NB
        ss = np.where(s_loc < RA, s_core * RA + s_loc,
                      s_core * RB + (s_loc - RA))
    else:
        ss = np.where(ss < VH, ss, ss - VH)
    key = (sc * NT + tc) * 2 + hc
    bounds = np.searchsorted(key, np.arange(NCORES * NT * 2 + 1))
    lists = {}
    for k in range(NCORES):
        for t in range(NT):
            for h in (0, 1):
                j = (k * NT + t) * 2 + h
                i0, i1 = bounds[j], bounds[j + 1]
                lists[(k, t, h)] = (ss[i0:i1], ds[i0:i1] % P)
    # exact slot counts per (tile, half): max over cores
    KA = [max(max((len(lists[(k, t, 0)][0]) for k in range(NCORES))), 1)
          for t in range(NT)]
    KB = [max(max((len(lists[(k, t, 1)][0]) for k in range(NCORES))), 1)
          for t in range(NT)]
    KA = [(c + P - 1) // P for c in KA]
    KB = [(c + P - 1) // P for c in KB]

    # greedy groups of consecutive tiles, <= SLOTMAX slots each
    groups = []  # list of (t0, gn, SA, SB)
    t0 = 0
    while t0 < NT:
        gn = 1
        sa, sb = KA[t0], KB[t0]
        while (t0 + gn < NT
               and sa + sb + KA[t0 + gn] + KB[t0 + gn] <= SLOTMAX):
            sa += KA[t0 + gn]
            sb += KB[t0 + gn]
            gn += 1
        groups.append((t0, gn, sa, sb))
        t0 += gn
    TOT = sum(sa + sb for _, _, sa, sb in groups)

    per_core = []
    for k in range(NCORES):
        idxa_cols, idxb_cols, drr_cols = [], [], []
        dstrel = np.full((P, TOT), 128.0, np.float32)
        soff = 0
        for (g0, gn, sa, sb) in groups:
            fa = np.zeros(sa * P, np.int16)
            fb = np.zeros(sb * P, np.int16)
            drow = np.full((sa + sb) * P, 128.0, np.float32)
            aoff = 0
            boff = sa
            for tl in range(gn):
                t = g0 + tl
                for h in (0, 1):
                    s_arr, r_arr = lists[(k, t, h)]
                    n = len(s_arr)
                    if h == 0:
                        base = aoff * P
                        fa[base:base + n] = s_arr.astype(np.int16)
                        slot0 = aoff
                        aoff += KA[t]
                    else:
                        base = (boff - sa) * P
                        fb[base:base + n] = s_arr.astype(np.int16)
                        slot0 = boff
                        boff += KB[t]
                    for i in range(n):
                        dstrel[i % P, soff + slot0 + i // P] = r_arr[i]
                        drow[(slot0 + i // P) * P + i % P] = r_arr[i]
            idxa_cols.append(_wrap_idx(fa))
            idxb_cols.append(_wrap_idx(fb))
            drr_cols.append(drow)
            soff += sa + sb
        per_core.append(dict(
            idxa=np.concatenate(idxa_cols, axis=1),
            idxb=np.concatenate(idxb_cols, axis=1),
            dstrel=dstrel.astype(ml_dtypes.bfloat16),
            dstrelr=np.concatenate(drr_cols)[None, :].astype(ml_dtypes.bfloat16),
        ))
    return groups, KA, KB, TOT, per_core, NB, NT


# --------------------------------------------------------------------------
# device kernel
# --------------------------------------------------------------------------

def apx(base_ap, pairs, extra_offset=0):
    return bass.AP(base_ap.tensor, base_ap.offset + extra_offset,
                   [list(p) for p in pairs])


def build_kernel(cfg, groups, KA, KB, TOT, NB, NT):
    F, H, DH, OUT = cfg["F"], cfg["H"], cfg["DH"], cfg["OUT"]
    ROW, TSPLIT = cfg["ROW"], cfg["TSPLIT"]
    SLOTMAX, NQ = cfg["SLOTMAX"], cfg["NQ"]
    SPLIT2 = cfg.get("SPLIT2", 1)
    FC = F // P
    NPAD = NB * NCORES
    RA = TSPLIT * P
    RB = NB - RA
    AluOp = mybir.AluOpType
    Act = mybir.ActivationFunctionType
    ABL = cfg.get("ABL", 5)

    nc = bacc.Bacc("TRN2", target_bir_lowering=False, debug=False,
                   num_devices=NCORES, num_swdge_queues=NQ)

    # ---- I/O ----
    xt_in = nc.declare_dram_parameter("xt", [FC, P, NB], BF16, isOutput=False)
    wf_in = [nc.declare_dram_parameter(f"wfull{l + 1}", [FC, P, 268], BF16,
                                       isOutput=False) for l in range(2)]
    wct_in = nc.declare_dram_parameter("wct", [FC, P, OUT], BF16, isOutput=False)
    gv_in = [nc.declare_dram_parameter(f"gvec{l + 1}", [P, FC], F32,
                                       isOutput=False) for l in range(2)]
    bv_in = [nc.declare_dram_parameter(f"bvec{l + 1}", [P, FC], F32,
                                       isOutput=False) for l in range(2)]
    bc_in = nc.declare_dram_parameter("bc_rep", [P, OUT], F32, isOutput=False)
    SA_tot = sum(sa for _, _, sa, _ in groups)
    SB_tot = sum(sb for _, _, _, sb in groups)
    idxa_in = nc.declare_dram_parameter("idxa", [P, SA_tot * 8], I16,
                                        isOutput=False)
    idxb_in = nc.declare_dram_parameter("idxb", [P, SB_tot * 8], I16,
                                        isOutput=False)
    ident_in = nc.declare_dram_parameter("ident", [P, P], F32, isOutput=False)
    iota_in = nc.declare_dram_parameter("iotarow", [P, P], BF16, isOutput=False)
    iotac_in = nc.declare_dram_parameter("iotacol", [P, P], F32, isOutput=False)
    ones1_in = nc.declare_dram_parameter("ones1", [1, P], BF16, isOutput=False)
    dstrel_in = nc.declare_dram_parameter("dstrel", [P, TOT], BF16,
                                          isOutput=False)
    drr_in = nc.declare_dram_parameter("dstrelr", [1, TOT * P], BF16,
                                       isOutput=False)
    out_ext = nc.declare_dram_parameter("out", [NB, OUT], F32, isOutput=True)

    if SPLIT2:
        haug_ownA = [nc.dram_tensor(f"haug_ownA{l}", [RA, ROW], BF16)
                     for l in (0, 1)]
        haug_ownB = [nc.dram_tensor(f"haug_ownB{l}", [RB, ROW], BF16)
                     for l in (0, 1)]
        haug_allA = [nc.dram_tensor(f"haug_allA{l}", [NCORES, RA, ROW], BF16,
                                    addr_space="Shared") for l in (0, 1)]
        haug_allB = [nc.dram_tensor(f"haug_allB{l}", [NCORES, RB, ROW], BF16,
                                    addr_space="Shared") for l in (0, 1)]
    else:
        haug_own = [nc.dram_tensor(f"haug_own{l}", [NB, ROW], BF16)
                    for l in (0, 1)]
        haug_all = [nc.dram_tensor(f"haug_all{l}", [NCORES, NB, ROW], BF16,
                                   addr_space="Shared") for l in (0, 1)]


    with tile.TileContext(nc) as tc:
        with (
            tc.tile_pool(name="const", bufs=1) as cpool,
            tc.tile_pool(name="persist", bufs=1) as ppool,
            tc.tile_pool(name="work", bufs=2) as wpool,
            tc.tile_pool(name="works", bufs=4) as spool,
            tc.tile_pool(name="gath", bufs=2) as gpool,
            tc.tile_pool(name="psmm", bufs=2, space="PSUM") as pspool,
            tc.tile_pool(name="psacc", bufs=3, space="PSUM") as accpool,
            tc.tile_pool(name="pstr", bufs=1, space="PSUM") as ptrpool,
            tc.tile_pool(name="pssd", bufs=2, space="PSUM") as sdpool,
        ):
            # ---- constants ----
            ident = cpool.tile([P, P], F32)
            nc.sync.dma_start(out=ident[:, :], in_=ident_in[:, :])
            iota_bf = cpool.tile([P, P], BF16)
            nc.sync.dma_start(out=iota_bf[:, :], in_=iota_in[:, :])
            iotac = cpool.tile([P, P], F32)
            nc.sync.dma_start(out=iotac[:, :], in_=iotac_in[:, :])
            ones1 = cpool.tile([1, P], BF16)
            nc.sync.dma_start(out=ones1[:, :], in_=ones1_in[:, :])
            wf_sb = [cpool.tile([P, FC, 268], BF16, tag=f"wf{l}", name=f"wf{l}")
                     for l in range(2)]
            for l in range(2):
                nc.sync.dma_start(out=wf_sb[l][:, :, :],
                                  in_=wf_in[l].rearrange("c p n -> p c n"))
            wct_sb = cpool.tile([P, FC, OUT], BF16)
            nc.sync.dma_start(out=wct_sb[:, :, :],
                              in_=wct_in.rearrange("c p n -> p c n"))
            gv_sb = [cpool.tile([P, FC], F32, tag=f"gv{l}", name=f"gv{l}")
                     for l in range(2)]
            bv_sb = [cpool.tile([P, FC], F32, tag=f"bv{l}", name=f"bv{l}")
                     for l in range(2)]
            for l in range(2):
                nc.sync.dma_start(out=gv_sb[l][:, :], in_=gv_in[l][:, :])
                nc.sync.dma_start(out=bv_sb[l][:, :], in_=bv_in[l][:, :])
            bc_sb = cpool.tile([P, OUT], F32)
            nc.sync.dma_start(out=bc_sb[:, :], in_=bc_in[:, :])
            dstrel_sb = cpool.tile([P, TOT], BF16)
            nc.sync.dma_start(out=dstrel_sb[:, :], in_=dstrel_in[:, :])
            SA_tot_ = sum(sa for _, _, sa, _ in groups)
            SB_tot_ = sum(sb for _, _, _, sb in groups)
            idxa_sb = cpool.tile([P, SA_tot_ * 8], I16)
            nc.sync.dma_start(out=idxa_sb[:, :], in_=idxa_in[:, :])
            idxb_sb = cpool.tile([P, SB_tot_ * 8], I16)
            nc.sync.dma_start(out=idxb_sb[:, :], in_=idxb_in[:, :])

            # buf0 = x (layer-1 input, never overwritten)
            # buf1 = layer-1 edge output (layer-2 input)
            xt_sb = [ppool.tile([P, FC, NB], BF16, tag=f"xt{l}", name=f"xt{l}")
                     for l in range(2)]
            nc.sync.dma_start(out=xt_sb[0][:, :, :],
                              in_=xt_in.rearrange("c p n -> p c n"))
            sdst_sb = ppool.tile([P, NT * H], BF16)

            for rep_ in range(cfg.get("REPEAT", 1)):
                for layer in (0, 1):
                    wfl = wf_sb[layer]
                    xt = xt_sb[layer]

                    # ---- node GEMM -> haug_own halves + local s_dst ----
                    for t in range(NT):
                        ps = pspool.tile([P, 512], F32, tag="mm")
                        for kc in range(FC):
                            nc.tensor.matmul(
                                ps[:, 0:268],
                                lhsT=xt[:, kc, t * P:(t + 1) * P],
                                rhs=wfl[:, kc, :],
                                start=(kc == 0), stop=(kc == FC - 1),
                            )
                        stg = spool.tile([P, ROW], BF16, tag="gemmout")
                        nc.scalar.copy(stg[:, 0:268], ps[:, 0:268])
                        nc.vector.memset(stg[:, 268:ROW], 0.0)
                        if not SPLIT2:
                            nc.sync.dma_start(
                                out=haug_own[layer][t * P:(t + 1) * P, :],
                                in_=stg[:, :])
                        elif t < TSPLIT:
                            nc.sync.dma_start(
                                out=haug_ownA[layer][t * P:(t + 1) * P, :],
                                in_=stg[:, :])
                        else:
                            t2 = t - TSPLIT
                            nc.sync.dma_start(
                                out=haug_ownB[layer][t2 * P:(t2 + 1) * P, :],
                                in_=stg[:, :])
                        nc.vector.tensor_copy(sdst_sb[:, t * H:(t + 1) * H],
                                              ps[:, 264:268])
                        if SPLIT2 and t == TSPLIT - 1:
                            # ---- share first half of the node table ----
                            if ABL == 4:
                                nc.sync.dma_start(
                                    out=haug_allA[layer][0, :, :],
                                    in_=haug_ownA[layer][:, :])
                            else:
                                nc.gpsimd.collective_compute(
                                    "AllGather", AluOp.bypass,
                                    replica_groups=[list(range(NCORES))],
                                    ins=[haug_ownA[layer][:, :]],
                                    outs=[haug_allA[layer][:, :, :]],
                                )
                    # ---- share second half (or, SPLIT2=0, everything) ----
                    if SPLIT2:
                        c_in, c_out = haug_ownB[layer], haug_allB[layer]
                    else:
                        c_in, c_out = haug_own[layer], haug_all[layer]
                    if ABL == 4:
                        nc.sync.dma_start(out=c_out[0, :, :], in_=c_in[:, :])
                    else:
                        nc.gpsimd.collective_compute(
                            "AllGather", AluOp.bypass,
                            replica_groups=[list(range(NCORES))],
                            ins=[c_in[:, :]],
                            outs=[c_out[:, :, :]],
                        )
                    if SPLIT2:
                        hflatA = haug_allA[layer].rearrange("c n d -> (c n) d")
                        hflatB = haug_allB[layer].rearrange("c n d -> (c n) d")
                    else:
                        hfl = haug_all[layer].rearrange("c n d -> (c n) d")
                        VH = cfg["VH"]
                        hflatA = hfl[0:VH, :]
                        hflatB = hfl[VH:NPAD, :]
                    if ABL == 1:
                        if layer == 0:
                            nc.vector.memset(xt_sb[1][:, :, :], 0.1)
                        continue

                    # ---- edge phase ----
                    offa = offb = offs = 0
                    for gi, (g0, gn, sa, sb) in enumerate(groups):
                        S = sa + sb
                        drr_t = wpool.tile([1, SLOTMAX * P], BF16, tag="drr")
                        nc.sync.dma_start(
                            out=drr_t[:, 0:S * P],
                            in_=drr_in[:, offs * P:(offs + S) * P])

                        gat = gpool.tile([P, SLOTMAX, ROW], BF16, tag="gat")
                        if ABL == 2:
                            nc.vector.memset(gat[:, :, :], 0.05)
                            nc.vector.tensor_copy(gat[:, 0:1, 0:8],
                                                  idxa_sb[:, 0:8])
                        else:
                            nc.gpsimd.dma_gather(
                                out_ap=gat[:, 0:sa, :], in_ap=hflatA[:, :],
                                idxs_ap=idxa_sb[:, offa:offa + sa * 8],
                                num_idxs=sa * P,
                                num_idxs_reg=sa * P, elem_size=ROW,
                                single_packet=False,
                                queue_num=(2 * gi) % NQ)
                            nc.gpsimd.dma_gather(
                                out_ap=gat[:, sa:S, :], in_ap=hflatB[:, :],
                                idxs_ap=idxb_sb[:, offb:offb + sb * 8],
                                num_idxs=sb * P,
                                num_idxs_reg=sb * P, elem_size=ROW,
                                single_packet=False,
                                queue_num=(2 * gi + 1) % NQ)
                        offa += sa * 8
                        offb += sb * 8

                        # one-hot [128e, S, 128n]
                        oh = wpool.tile([P, SLOTMAX, P], BF16, tag="oh")
                        dr = dstrel_sb[:, offs:offs + S]
                        iota_ap = iota_bf[:, :]
                        nc.vector.tensor_tensor(
                            out=oh[:, 0:S, :],
                            in0=apx(iota_ap, [iota_ap.ap[0], [0, S], [1, P]]),
                            in1=dr.to_broadcast([P, S, P]),
                            op=AluOp.is_equal)

                        # transposed one-hot: ohT[n, s, e] = (n == dstrel[s,e])
                        ohT = wpool.tile([P, SLOTMAX, P], BF16, tag="ohT")
                        nch = (S + 3) // 4 if ABL != 10 else 0
                        for c in range(nch):
                            cols = min(4, S - 4 * c) * P
                            bc_ps = pspool.tile([P, 512], F32, tag="mm",
                                                name="bc_ps")
                            nc.tensor.matmul(
                                bc_ps[:, 0:cols],
                                lhsT=ones1[:, :],
                                rhs=drr_t[:, 4 * c * P:4 * c * P + cols],
                                start=True, stop=True)
                            iotac_ap = iotac[:, :]
                            nc.vector.tensor_tensor(
                                out=ohT[:, 4 * c:4 * c + cols // P, :],
                                in0=apx(iotac_ap,
                                        [iotac_ap.ap[0], [0, cols // P],
                                         [1, P]]),
                                in1=apx(bc_ps[:, :],
                                        [bc_ps[:, :].ap[0], [P, cols // P],
                                         [1, P]]),
                                op=AluOp.is_equal)

                        # per-edge s_dst via ohT matmul against own s_dst
                        sdacc = sdpool.tile([P, SLOTMAX, H], F32, tag="sdacc")
                        if ABL != 10:
                            sl = 0
                            for h_ in (0, 1):
                                for tl in range(gn):
                                    t = g0 + tl
                                    for _ in range(KA[t] if h_ == 0 else KB[t]):
                                        nc.tensor.matmul(
                                            sdacc[:, sl, :],
                                            lhsT=ohT[:, sl, :],
                                            rhs=sdst_sb[:, t * H:(t + 1) * H],
                                            start=True, stop=True)
                                        sl += 1

                        # e = lrelu(ssrc + sdst); w = exp(e)
                        ef = spool.tile([P, SLOTMAX, H], F32, tag="ef")
                        if ABL == 10:
                            nc.vector.tensor_scalar(
                                out=ef[:, 0:S, :], in0=gat[:, 0:S, 260:264],
                                scalar1=1.0, scalar2=None, op0=AluOp.mult)
                        else:
                            nc.vector.tensor_tensor(
                                out=ef[:, 0:S, :], in0=gat[:, 0:S, 260:264],
                                in1=sdacc[:, 0:S, :], op=AluOp.add)
                        efs = spool.tile([P, SLOTMAX, H], F32, tag="efs")
                        nc.vector.tensor_scalar(
                            out=efs[:, 0:S, :], in0=ef[:, 0:S, :], scalar1=0.2,
                            scalar2=None, op0=AluOp.mult)
                        nc.vector.tensor_tensor(
                            out=ef[:, 0:S, :], in0=ef[:, 0:S, :],
                            in1=efs[:, 0:S, :], op=AluOp.max)
                        wexp = spool.tile([P, SLOTMAX, H], BF16, tag="wexp")
                        nc.scalar.activation(wexp[:, 0:S, :], ef[:, 0:S, :],
                                             Act.Exp)

                        # scale gathered rows by w; cols 256:260 = w itself
                        hts = wpool.tile([P, SLOTMAX, 260], BF16, tag="hts")
                        if ABL == 12:
                            nc.vector.tensor_copy(hts[:, 0:1, :],
                                                  gat[:, 0:1, 0:260])
                        else:
                            nc.vector.tensor_tensor(
                                out=hts[:, 0:S, 0:256].rearrange(
                                    "p s (h d) -> p s h d", h=H),
                                in0=gat[:, 0:S, 0:256].rearrange(
                                    "p s (h d) -> p s h d", h=H),
                                in1=wexp[:, 0:S, :].to_broadcast([P, S, H, DH]),
                                op=AluOp.mult)
                            nc.vector.tensor_copy(hts[:, 0:S, 256:260],
                                                  wexp[:, 0:S, :])

                        # scatter + normalize + BN/ELU per tile
                        aoff = 0
                        boff = sa
                        for tl in range(gn):
                            t = g0 + tl
                            slots = ([aoff + s for s in range(KA[t])] +
                                     [boff + s for s in range(KB[t])])
                            aoff += KA[t]
                            boff += KB[t]
                            if ABL == 11:
                                slots = slots[:1]
                            K_t = len(slots)
                            acc = accpool.tile([P, 260], F32, tag="acc")
                            rsrc = gat if ABL == 12 else hts
                            for j, slx in enumerate(slots):
                                nc.tensor.matmul(
                                    acc[:, :],
                                    lhsT=oh[:, slx, :],
                                    rhs=rsrc[:, slx, 0:260],
                                    start=(j == 0), stop=(j == K_t - 1),
                                )
                            tmax = spool.tile([P, H], F32, tag="tmax")
                            nc.vector.tensor_scalar(
                                out=tmax[:, :], in0=acc[:, 256:260],
                                scalar1=1e-9, scalar2=None, op0=AluOp.max)
                            rec = spool.tile([P, H], F32, tag="rec")
                            nc.vector.reciprocal(rec[:, :], tmax[:, :])
                            zsb = spool.tile([P, F], F32, tag="zsb")
                            nc.vector.tensor_tensor(
                                out=zsb[:, :].rearrange("p (h d) -> p h d",
                                                        h=H),
                                in0=acc[:, 0:256].rearrange("p (h d) -> p h d",
                                                            h=H),
                                in1=rec[:, :].to_broadcast([P, H, DH]),
                                op=AluOp.mult)
                            # transpose + BN + ELU per feature chunk
                            cls_in = spool.tile([P, FC, P], BF16, tag="clsin")
                            for fc in range(FC):
                                pst = ptrpool.tile([P, P], F32, tag="ptr")
                                nc.tensor.transpose(
                                    pst[:, :], zsb[:, fc * P:(fc + 1) * P],
                                    ident[:, :])
                                ybn = spool.tile([P, P], F32, tag="ybn")
                                nc.scalar.activation(
                                    ybn[:, :], pst[:, :], Act.Identity,
                                    bias=bv_sb[layer][:, fc:fc + 1],
                                    scale=gv_sb[layer][:, fc:fc + 1])
                                ey = spool.tile([P, P], F32, tag="ey")
                                nc.scalar.activation(ey[:, :], ybn[:, :],
                                                     Act.Exp)
                                nc.vector.tensor_scalar(
                                    out=ey[:, :], in0=ey[:, :], scalar1=1.0,
                                    scalar2=0.0, op0=AluOp.subtract,
                                    op1=AluOp.min)
                                nc.vector.tensor_scalar(
                                    out=ybn[:, :], in0=ybn[:, :], scalar1=0.0,
                                    scalar2=None, op0=AluOp.max)
                                if layer == 0:
                                    nc.vector.tensor_tensor(
                                        out=xt_sb[1][:, fc, t * P:(t + 1) * P],
                                        in0=ey[:, :], in1=ybn[:, :],
                                        op=AluOp.add)
                                else:
                                    nc.vector.tensor_tensor(
                                        out=cls_in[:, fc, :],
                                        in0=ey[:, :], in1=ybn[:, :],
                                        op=AluOp.add)
                            if layer == 1:
                                # fused classifier for this tile
                                cps = pspool.tile([P, 512], F32, tag="mm",
                                                  name="cps")
                                for kc in range(FC):
                                    nc.tensor.matmul(
                                        cps[:, 0:OUT],
                                        lhsT=cls_in[:, kc, :],
                                        rhs=wct_sb[:, kc, :],
                                        start=(kc == 0), stop=(kc == FC - 1),
                                    )
                                ob = spool.tile([P, OUT], F32, tag="ob")
                                nc.vector.tensor_tensor(
                                    out=ob[:, :], in0=cps[:, 0:OUT],
                                    in1=bc_sb[:, :], op=AluOp.add)
                                nc.sync.dma_start(
                                    out=out_ext[t * P:(t + 1) * P, :],
                                    in_=ob[:, :])
                        offs += S

                if ABL == 1:
                    # classifier over (garbage) layer-1 buffer, timing only
                    for t in range(NT):
                        cps = pspool.tile([P, 512], F32, tag="mm", name="cps1")
                        for kc in range(FC):
                            nc.tensor.matmul(
                                cps[:, 0:OUT],
                                lhsT=xt_sb[1][:, kc, t * P:(t + 1) * P],
                                rhs=wct_sb[:, kc, :],
                                start=(kc == 0), stop=(kc == FC - 1),
                            )
                        ob = spool.tile([P, OUT], F32, tag="ob")
                        nc.vector.tensor_tensor(out=ob[:, :], in0=cps[:, 0:OUT],
                                                in1=bc_sb[:, :], op=AluOp.add)
                        nc.sync.dma_start(out=out_ext[t * P:(t + 1) * P, :],
                                          in_=ob[:, :])

    nc.compile()
    return nc


# --------------------------------------------------------------------------
# entry point
# --------------------------------------------------------------------------

def kernel(x, edge_index, W1, a_src1, a_dst1, bn1_gamma, bn1_beta, bn1_mean,
           bn1_var, W2, a_src2, a_dst2, bn2_gamma, bn2_beta, bn2_mean, bn2_var,
           Wc, bc, _cfg=None, _run_kwargs=None, _bench=0):
    cfg = dict(CFG)
    if _cfg:
        cfg.update(_cfg)
    N, F, OUT = cfg["N"], cfg["F"], cfg["OUT"]
    FC = F // P

    if cfg.get("PERM", 1):
        pos = balance_perm(cfg, edge_index)
        edge_index = pos[np.asarray(edge_index, np.int64)]
    else:
        pos = None
    groups, KA, KB, TOT, per_core, NB, NT = prep_edges(cfg, edge_index)
    nc = build_kernel(cfg, groups, KA, KB, TOT, NB, NT)

    wfull1 = _wfull(W1, a_src1, a_dst1)
    wfull2 = _wfull(W2, a_src2, a_dst2)
    wct = np.ascontiguousarray(np.asarray(Wc, np.float64).T).astype(
        ml_dtypes.bfloat16)
    g1, b1 = _bn_consts(bn1_gamma, bn1_beta, bn1_mean, bn1_var)
    g2, b2 = _bn_consts(bn2_gamma, bn2_beta, bn2_mean, bn2_var)
    bc_rep = np.tile(np.asarray(bc, np.float32)[None, :], (P, 1))

    xpad = np.zeros((NB * NCORES, F), np.float32)
    if pos is not None:
        xpad[pos[:N]] = np.asarray(x, np.float32)
    else:
        xpad[:N] = np.asarray(x, np.float32)
    xt = np.ascontiguousarray(xpad.T).astype(ml_dtypes.bfloat16)  # [F, NPAD]

    in_maps = []
    for k in range(NCORES):
        xk = xt[:, k * NB:(k + 1) * NB].reshape(FC, P, NB)
        in_maps.append(dict(
            xt=np.ascontiguousarray(xk),
            wfull1=np.ascontiguousarray(wfull1.reshape(FC, P, 268)),
            wfull2=np.ascontiguousarray(wfull2.reshape(FC, P, 268)),
            wct=np.ascontiguousarray(wct.reshape(FC, P, OUT)),
            gvec1=g1, bvec1=b1, gvec2=g2, bvec2=b2, bc_rep=bc_rep,
            idxa=per_core[k]["idxa"], idxb=per_core[k]["idxb"],
            dstrel=per_core[k]["dstrel"], dstrelr=per_core[k]["dstrelr"],
            ident=np.eye(P, dtype=np.float32),
            iotarow=np.tile(np.arange(P, dtype=np.float32)[None, :],
                            (P, 1)).astype(ml_dtypes.bfloat16),
            iotacol=np.ascontiguousarray(
                np.tile(np.arange(P, dtype=np.float32)[:, None], (1, P))),
            ones1=np.ones((1, P), np.float32).astype(ml_dtypes.bfloat16),
        ))

    res = run_bass_kernel_spmd(nc, in_maps, list(range(NCORES)),
                               **(_run_kwargs or {}))
    out = np.concatenate([res.results[k]["out"] for k in range(NCORES)], axis=0)
    if pos is not None:
        out = out[pos[:N]]
    out = out[:N].astype(np.float32)
    if _bench:
        ns = _bench_pjrt(nc, in_maps, _bench)
        return out, ns
    if _run_kwargs is not None:
        return out, res
    return out


def _bench_pjrt(nc, in_maps, iters):
    """Median per-iteration wall time (ns) of the NEFF execution via PJRT,
    device-resident inputs, back-to-back async dispatch."""
    import time
    import jax
    import jax.numpy as jnp
    from jax.sharding import Mesh, PartitionSpec
    from jax.experimental.shard_map import shard_map
    from concourse import bass2jax
    from concourse.bass2jax import _bass_exec_p, partition_id_tensor
    import concourse.mybir as mybir

    n_cores = len(in_maps)
    partition_name = nc.partition_id_tensor.name if nc.partition_id_tensor else None
    in_names, out_names, out_avals, zero_outs = [], [], [], []
    for alloc in nc.m.functions[0].allocations:
        if not isinstance(alloc, mybir.MemoryLocationSet):
            continue
        name = alloc.memorylocations[0].name
        if alloc.kind == "ExternalInput":
            if name != partition_name:
                in_names.append(name)
        elif alloc.kind == "ExternalOutput":
            shape = list(alloc.tensor_shape)
            dt = mybir.dt.np(alloc.dtype)
            out_avals.append(jax.core.ShapedArray(shape, dt))
            out_names.append(name)
            zero_outs.append(np.zeros(shape, dt))
    n_params = len(in_names)
    n_outs = len(out_names)
    in_names.extend(out_names)
    if partition_name is not None:
        in_names.append(partition_name)

    def _body(*args):
        operands = list(args)
        if partition_name is not None:
            operands.append(partition_id_tensor())
        return tuple(_bass_exec_p.bind(
            *operands, out_avals=tuple(out_avals), in_names=tuple(in_names),
            out_names=tuple(out_names), lowering_input_output_aliases=(),
            sim_require_finite=True, sim_require_nnan=True, nc=nc))

    devices = jax.devices()[:n_cores]
    mesh = Mesh(np.asarray(devices), ("core",))
    sharded = jax.jit(
        shard_map(_body, mesh=mesh,
                  in_specs=(PartitionSpec("core"),) * (n_params + n_outs),
                  out_specs=(PartitionSpec("core"),) * n_outs,
                  check_rep=False),
        donate_argnums=(), keep_unused=True)
    per_core = [[np.asarray(m[name]) for name in in_names[:n_params]]
                for m in in_maps]
    concat_in = [np.concatenate([per_core[c][i] for c in range(n_cores)], axis=0)
                 for i in range(n_params)]
    from jax.sharding import NamedSharding
    sh = NamedSharding(mesh, PartitionSpec("core"))
    dev_in = [jax.device_put(a, sh) for a in concat_in]
    zshapes = [(n_cores * z.shape[0], *z.shape[1:]) for z in zero_outs]
    zdtypes = [z.dtype for z in zero_outs]

    dev_zeros = [jax.device_put(np.zeros(s_, d_), sh)
                 for s_, d_ in zip(zshapes, zdtypes)]

    def one_iter():
        return sharded(*dev_in, *dev_zeros)

    jax.block_until_ready(one_iter())
    times = []
    for _ in range(5):
        t0 = time.perf_counter()
        outs = [one_iter() for _ in range(iters)]
        jax.block_until_ready(outs[-1])
        times.append((time.perf_counter() - t0) / iters * 1e9)
    return min(times)


# revision 23
# speedup vs baseline: 1.7055x; 1.1290x over previous
"""GAT (2-layer, 4-head) Trainium2 kernel over 8 NeuronCores — v2.

Strategy:
  * Edges sorted by dst, dst-range partitioned across the 8 cores (each core
    owns N/8 node rows and fully computes their output -> no output
    all-reduce, softmax stats stay core-local).
  * Per layer: node GEMM is data-parallel over the owned node range, the
    augmented node table (h | ones | s_src | s_dst) is AllGathered to every
    core's HBM, then the edge phase gathers h[src] rows with dma_gather and
    performs the segment softmax + weighted scatter-add as one-hot matmuls
    accumulated in PSUM.
  * v2: the per-edge s_dst values come from a transposed one-hot matmul
    against the tile's own s_dst vector (built via a rank-1 PE broadcast of
    the host-known dstrel row + DVE is_equal) instead of a second dma_gather;
    per-(tile,half) slot counts are exact (max over cores) instead of a
    global max; gathers round-robin over 4 SWDGE queues; the classifier is
    fused into layer 2's edge phase.
  * int16 gather indices cap at 32767, so the node table is addressed as two
    halves (rows < VH and the rest) with per-tile A/B edge slot groups.
"""

import sys

if "/opt/trn_rl_repo" not in sys.path:
    sys.path.insert(0, "/opt/trn_rl_repo")

import ml_dtypes
import numpy as np

import concourse.bacc as bacc
import concourse.bass as bass
import concourse.mybir as mybir
import concourse.tile as tile
from concourse.bass_utils import run_bass_kernel_spmd

BF16 = mybir.dt.bfloat16
F32 = mybir.dt.float32
I16 = mybir.dt.int16

NCORES = 8
P = 128

CFG = dict(
    N=50000,
    E=500000,
    F=256,      # feature width (in = hid = 256)
    H=4,
    DH=64,
    OUT=64,
    ROW=384,    # padded bf16 row length of node table (768B, %256B)
    SLOTMAX=28,  # max 128-edge slots per gather group
    NQ=4,       # SWDGE queues for gathers
)


# --------------------------------------------------------------------------
# host-side preparation
# --------------------------------------------------------------------------

def _head_matrix(a):
    """[H, DH] -> block diagonal [F, H] so that s = h @ A."""
    H, DH = np.asarray(a).shape
    A = np.zeros((H * DH, H), np.float64)
    for h in range(H):
        A[h * DH:(h + 1) * DH, h] = np.asarray(a, np.float64)[h]
    return A


def _wfull(W, a_src, a_dst):
    """[W^T | zeros | W^T@Asrc | W^T@Adst] as [F, 268] bf16."""
    W = np.asarray(W, np.float64)
    F = W.shape[1]
    Wt = W.T
    Bs = Wt @ _head_matrix(a_src)
    Bd = Wt @ _head_matrix(a_dst)
    out = np.zeros((F, 268), np.float64)
    out[:, :W.shape[0]] = Wt
    out[:, 260:264] = Bs
    out[:, 264:268] = Bd
    return out.astype(ml_dtypes.bfloat16)


def _bn_consts(gamma, beta, mean, var, eps=1e-5):
    gamma = np.asarray(gamma, np.float64)
    beta = np.asarray(beta, np.float64)
    mean = np.asarray(mean, np.float64)
    var = np.asarray(var, np.float64)
    g = gamma / np.sqrt(var + eps)
    b = beta - mean * g
    F = gamma.shape[0]
    return (
        np.ascontiguousarray(g.reshape(F // P, P).T.astype(np.float32)),
        np.ascontiguousarray(b.reshape(F // P, P).T.astype(np.float32)),
    )


def _wrap_idx(flat):
    """int16 position array -> dma_gather wrapped layout [128, len//16]."""
    n = len(flat)
    assert n % 16 == 0
    w = np.zeros((P, n // 16), np.int16)
    w[:16, :] = np.asarray(flat, np.int16).reshape(-1, 16).T
    w[16:, :] = np.tile(w[:16, :], (7, 1))
    return w


def balance_perm(cfg, edge_index):
    """Permute nodes within each core's range so per-tile (A,B) in-degree
    sums are balanced -> fewer 128-edge gather slots. Returns pos[id]."""
    N = cfg["N"]
    NB = ((N + NCORES - 1) // NCORES + P - 1) // P * P
    NT = NB // P
    RA = (NT // 2) * P
    NPAD = NB * NCORES
    src = np.asarray(edge_index[0], np.int64)
    dst = np.asarray(edge_index[1], np.int64)
    TSPLIT = NT // 2
    # nodes never change half (A = local < RA), so the (inA, inB) in-degree
    # labels stay exact and one packing pass is sufficient
    a_lab = (src % NB) < RA
    inA = np.bincount(dst[a_lab], minlength=NPAD).astype(np.int64)
    inB = np.bincount(dst[~a_lab], minlength=NPAD).astype(np.int64)
    base = 5 * P
    # fixed overflow-tile indices shared by all cores (per-tile slot count
    # is a max over cores): per half-group, A overflow at its low tiles,
    # B overflow at its high tiles
    capA_t = np.full(NT, base, np.int64)
    capB_t = np.full(NT, base, np.int64)
    for t0, nbin in ((0, TSPLIT), (TSPLIT, NT - TSPLIT)):
        needA = needB = 0
        for k in range(NCORES):
            ids = np.arange(k * NB + t0 * P, k * NB + (t0 + nbin) * P)
            needA = max(needA, (inA[ids].sum() - nbin * base + P - 1) // P)
            needB = max(needB, (inB[ids].sum() - nbin * base + P - 1) // P)
        nExA = int(max(needA, 0)) + 2
        nExB = int(max(needB, 0)) + 2
        capA_t[t0:t0 + nExA] += P
        capB_t[t0 + nbin - nExB:t0 + nbin] += P
    pos = np.empty(NPAD, np.int64)
    for k in range(NCORES):
        for t0, nbin in ((0, TSPLIT), (TSPLIT, NT - TSPLIT)):
            lo = k * NB + t0 * P
            ids = np.arange(lo, lo + nbin * P)
            capA = capA_t[t0:t0 + nbin]
            capB = capB_t[t0:t0 + nbin]
            tot = inA[ids] + inB[ids]
            order = np.argsort(-tot, kind="stable")
            sumA = np.zeros(nbin)
            sumB = np.zeros(nbin)
            cnt = np.zeros(nbin)
            lane = np.zeros(nbin, np.int64)
            fA = capA.astype(np.float64)
            fB = capB.astype(np.float64)
            for j in order:
                a, b = inA[ids[j]], inB[ids[j]]
                load = np.maximum((sumA + a) / fA, (sumB + b) / fB)
                load = np.maximum(load, (cnt + 1) / P) + (cnt >= P) * 1e9
                t = int(np.argmin(load))
                sumA[t] += a
                sumB[t] += b
                cnt[t] += 1
                pos[ids[j]] = lo + t * P + lane[t]
                lane[t] += 1
    return pos


def prep_edges(cfg, edge_index):
    """Sort/partition edges; exact per-(tile,half) slot counts (max over
    cores), greedy tile groups bounded by SLOTMAX slots."""
    N, SLOTMAX = cfg["N"], cfg["SLOTMAX"]
    NB = ((N + NCORES - 1) // NCORES + P - 1) // P * P
    NT = NB // P
    SPLIT2 = cfg.get("SPLIT2", 1)
    src = np.asarray(edge_index[0], np.int64)
    dst = np.asarray(edge_index[1], np.int64)
    core = dst // NB
    tilein = (dst % NB) // P
    if SPLIT2:
        # split the shared table at tile TSPLIT: A = local rows < RA, B =
        # rest; both halves must index within int16 across all cores
        TSPLIT = NT // 2
        RA = TSPLIT * P
        RB = NB - RA
        assert NCORES * RA <= 32768 and NCORES * RB <= 32768
        cfg["TSPLIT"] = TSPLIT
        half = ((src % NB) >= RA).astype(np.int64)
    else:
        VH = min(32768, NB * NCORES // 2)
        cfg["VH"] = VH
        cfg["TSPLIT"] = NT
        half = (src >= VH).astype(np.int64)
    order = np.lexsort((src, half, tilein, core))
    sc, tc, hc = core[order], tilein[order], half[order]
    ss, ds = src[order], dst[order]
    if SPLIT2:
        # remap src to half-table indices
        s_core = ss // NB
        s_loc = ss % NB
        ss = np.where(s_loc < RA, s_core * RA + s_loc,
                      s_core * RB + (s_loc - RA))
    else:
        ss = np.where(ss < VH, ss, ss - VH)
    key = (sc * NT + tc) * 2 + hc
    bounds = np.searchsorted(key, np.arange(NCORES * NT * 2 + 1))
    lists = {}
    for k in range(NCORES):
        for t in range(NT):
            for h in (0, 1):
                j = (k * NT + t) * 2 + h
                i0, i1 = bounds[j], bounds[j + 1]
                lists[(k, t, h)] = (ss[i0:i1], ds[i0:i1] % P)
    # exact slot counts per (tile, half): max over cores
    KA = [max(max((len(lists[(k, t, 0)][0]) for k in range(NCORES))), 1)
          for t in range(NT)]
    KB = [max(max((len(lists[(k, t, 1)][0]) for k in range(NCORES))), 1)
          for t in range(NT)]
    KA = [(c + P - 1) // P for c in KA]
    KB = [(c + P - 1) // P for c in KB]

    # greedy groups of consecutive tiles, <= SLOTMAX slots each
    groups = []  # list of (t0, gn, SA, SB)
    t0 = 0
    while t0 < NT:
        gn = 1
        sa, sb = KA[t0], KB[t0]
        while (t0 + gn < NT
               and sa + sb + KA[t0 + gn] + KB[t0 + gn] <= SLOTMAX):
            sa += KA[t0 + gn]
            sb += KB[t0 + gn]
            gn += 1
        groups.append((t0, gn, sa, sb))
        t0 += gn
    TOT = sum(sa + sb for _, _, sa, sb in groups)

    per_core = []
    for k in range(NCORES):
        idxa_cols, idxb_cols, drr_cols = [], [], []
        dstrel = np.full((P, TOT), 128.0, np.float32)
        soff = 0
        for (g0, gn, sa, sb) in groups:
            fa = np.zeros(sa * P, np.int16)
            fb = np.zeros(sb * P, np.int16)
            drow = np.full((sa + sb) * P, 128.0, np.float32)
            aoff = 0
            boff = sa
            for tl in range(gn):
                t = g0 + tl
                for h in (0, 1):
                    s_arr, r_arr = lists[(k, t, h)]
                    n = len(s_arr)
                    if h == 0:
                        base = aoff * P
                        fa[base:base + n] = s_arr.astype(np.int16)
                        slot0 = aoff
                        aoff += KA[t]
                    else:
                        base = (boff - sa) * P
                        fb[base:base + n] = s_arr.astype(np.int16)
                        slot0 = boff
                        boff += KB[t]
                    for i in range(n):
                        dstrel[i % P, soff + slot0 + i // P] = r_arr[i]
                        drow[(slot0 + i // P) * P + i % P] = r_arr[i]
            idxa_cols.append(_wrap_idx(fa))
            idxb_cols.append(_wrap_idx(fb))
            drr_cols.append(drow)
            soff += sa + sb
        per_core.append(dict(
            idxa=np.concatenate(idxa_cols, axis=1),
            idxb=np.concatenate(idxb_cols, axis=1),
            dstrel=dstrel.astype(ml_dtypes.bfloat16),
            dstrelr=np.concatenate(drr_cols)[None, :].astype(ml_dtypes.bfloat16),
        ))
    return groups, KA, KB, TOT, per_core, NB, NT


# --------------------------------------------------------------------------
# device kernel
# --------------------------------------------------------------------------

def apx(base_ap, pairs, extra_offset=0):
    return bass.AP(base_ap.tensor, base_ap.offset + extra_offset,
                   [list(p) for p in pairs])


def build_kernel(cfg, groups, KA, KB, TOT, NB, NT):
    F, H, DH, OUT = cfg["F"], cfg["H"], cfg["DH"], cfg["OUT"]
    ROW, TSPLIT = cfg["ROW"], cfg["TSPLIT"]
    SLOTMAX, NQ = cfg["SLOTMAX"], cfg["NQ"]
    SPLIT2 = cfg.get("SPLIT2", 1)
    FC = F // P
    NPAD = NB * NCORES
    RA = TSPLIT * P
    RB = NB - RA
    AluOp = mybir.AluOpType
    Act = mybir.ActivationFunctionType
    ABL = cfg.get("ABL", 5)

    nc = bacc.Bacc("TRN2", target_bir_lowering=False, debug=False,
                   num_devices=NCORES, num_swdge_queues=NQ)

    # ---- I/O ----
    xt_in = nc.declare_dram_parameter("xt", [FC, P, NB], BF16, isOutput=False)
    wf_in = [nc.declare_dram_parameter(f"wfull{l + 1}", [FC, P, 268], BF16,
                                       isOutput=False) for l in range(2)]
    wct_in = nc.declare_dram_parameter("wct", [FC, P, OUT], BF16, isOutput=False)
    gv_in = [nc.declare_dram_parameter(f"gvec{l + 1}", [P, FC], F32,
                                       isOutput=False) for l in range(2)]
    bv_in = [nc.declare_dram_parameter(f"bvec{l + 1}", [P, FC], F32,
                                       isOutput=False) for l in range(2)]
    bc_in = nc.declare_dram_parameter("bc_rep", [P, OUT], F32, isOutput=False)
    SA_tot = sum(sa for _, _, sa, _ in groups)
    SB_tot = sum(sb for _, _, _, sb in groups)
    idxa_in = nc.declare_dram_parameter("idxa", [P, SA_tot * 8], I16,
                                        isOutput=False)
    idxb_in = nc.declare_dram_parameter("idxb", [P, SB_tot * 8], I16,
                                        isOutput=False)
    ident_in = nc.declare_dram_parameter("ident", [P, P], F32, isOutput=False)
    iota_in = nc.declare_dram_parameter("iotarow", [P, P], BF16, isOutput=False)
    iotac_in = nc.declare_dram_parameter("iotacol", [P, P], F32, isOutput=False)
    ones1_in = nc.declare_dram_parameter("ones1", [1, P], BF16, isOutput=False)
    dstrel_in = nc.declare_dram_parameter("dstrel", [P, TOT], BF16,
                                          isOutput=False)
    drr_in = nc.declare_dram_parameter("dstrelr", [1, TOT * P], BF16,
                                       isOutput=False)
    out_ext = nc.declare_dram_parameter("out", [NB, OUT], F32, isOutput=True)

    if SPLIT2:
        haug_ownA = [nc.dram_tensor(f"haug_ownA{l}", [RA, ROW], BF16)
                     for l in (0, 1)]
        haug_ownB = [nc.dram_tensor(f"haug_ownB{l}", [RB, ROW], BF16)
                     for l in (0, 1)]
        haug_allA = [nc.dram_tensor(f"haug_allA{l}", [NCORES, RA, ROW], BF16,
                                    addr_space="Shared") for l in (0, 1)]
        haug_allB = [nc.dram_tensor(f"haug_allB{l}", [NCORES, RB, ROW], BF16,
                                    addr_space="Shared") for l in (0, 1)]
    else:
        haug_own = [nc.dram_tensor(f"haug_own{l}", [NB, ROW], BF16)
                    for l in (0, 1)]
        haug_all = [nc.dram_tensor(f"haug_all{l}", [NCORES, NB, ROW], BF16,
                                   addr_space="Shared") for l in (0, 1)]


    with tile.TileContext(nc) as tc:
        with (
            tc.tile_pool(name="const", bufs=1) as cpool,
            tc.tile_pool(name="persist", bufs=1) as ppool,
            tc.tile_pool(name="work", bufs=2) as wpool,
            tc.tile_pool(name="works", bufs=4) as spool,
            tc.tile_pool(name="gath", bufs=2) as gpool,
            tc.tile_pool(name="psmm", bufs=2, space="PSUM") as pspool,
            tc.tile_pool(name="psacc", bufs=2, space="PSUM") as accpool,
            tc.tile_pool(name="pssd", bufs=2, space="PSUM") as sdpool,
        ):
            # ---- constants ----
            ident = cpool.tile([P, P], F32)
            nc.sync.dma_start(out=ident[:, :], in_=ident_in[:, :])
            iota_bf = cpool.tile([P, P], BF16)
            nc.sync.dma_start(out=iota_bf[:, :], in_=iota_in[:, :])
            iotac = cpool.tile([P, P], F32)
            nc.sync.dma_start(out=iotac[:, :], in_=iotac_in[:, :])
            ones1 = cpool.tile([1, P], BF16)
            nc.sync.dma_start(out=ones1[:, :], in_=ones1_in[:, :])
            wf_sb = [cpool.tile([P, FC, 268], BF16, tag=f"wf{l}", name=f"wf{l}")
                     for l in range(2)]
            for l in range(2):
                nc.sync.dma_start(out=wf_sb[l][:, :, :],
                                  in_=wf_in[l].rearrange("c p n -> p c n"))
            wct_sb = cpool.tile([P, FC, OUT], BF16)
            nc.sync.dma_start(out=wct_sb[:, :, :],
                              in_=wct_in.rearrange("c p n -> p c n"))
            gv_sb = [cpool.tile([P, FC], F32, tag=f"gv{l}", name=f"gv{l}")
                     for l in range(2)]
            bv_sb = [cpool.tile([P, FC], F32, tag=f"bv{l}", name=f"bv{l}")
                     for l in range(2)]
            for l in range(2):
                nc.sync.dma_start(out=gv_sb[l][:, :], in_=gv_in[l][:, :])
                nc.sync.dma_start(out=bv_sb[l][:, :], in_=bv_in[l][:, :])
            bc_sb = cpool.tile([P, OUT], F32)
            nc.sync.dma_start(out=bc_sb[:, :], in_=bc_in[:, :])
            dstrel_sb = cpool.tile([P, TOT], BF16)
            nc.sync.dma_start(out=dstrel_sb[:, :], in_=dstrel_in[:, :])
            SA_tot_ = sum(sa for _, _, sa, _ in groups)
            SB_tot_ = sum(sb for _, _, _, sb in groups)
            idxa_sb = cpool.tile([P, SA_tot_ * 8], I16)
            nc.sync.dma_start(out=idxa_sb[:, :], in_=idxa_in[:, :])
            idxb_sb = cpool.tile([P, SB_tot_ * 8], I16)
            nc.sync.dma_start(out=idxb_sb[:, :], in_=idxb_in[:, :])

            # buf0 = x (layer-1 input, never overwritten)
            # buf1 = layer-1 edge output (layer-2 input)
            xt_sb = [ppool.tile([P, FC, NB], BF16, tag=f"xt{l}", name=f"xt{l}")
                     for l in range(2)]
            nc.sync.dma_start(out=xt_sb[0][:, :, :],
                              in_=xt_in.rearrange("c p n -> p c n"))
            sdst_sb = ppool.tile([P, NT * H], BF16)

            for rep_ in range(cfg.get("REPEAT", 1)):
                for layer in (0, 1):
                    wfl = wf_sb[layer]
                    xt = xt_sb[layer]

                    # ---- node GEMM -> haug_own halves + local s_dst ----
                    for t in range(NT):
                        ps = pspool.tile([P, 512], F32, tag="mm")
                        for kc in range(FC):
                            nc.tensor.matmul(
                                ps[:, 0:268],
                                lhsT=xt[:, kc, t * P:(t + 1) * P],
                                rhs=wfl[:, kc, :],
                                start=(kc == 0), stop=(kc == FC - 1),
                            )
                        stg = spool.tile([P, ROW], BF16, tag="gemmout")
                        nc.scalar.copy(stg[:, 0:268], ps[:, 0:268])
                        nc.vector.memset(stg[:, 268:ROW], 0.0)
                        if not SPLIT2:
                            nc.sync.dma_start(
                                out=haug_own[layer][t * P:(t + 1) * P, :],
                                in_=stg[:, :])
                        elif t < TSPLIT:
                            nc.sync.dma_start(
                                out=haug_ownA[layer][t * P:(t + 1) * P, :],
                                in_=stg[:, :])
                        else:
                            t2 = t - TSPLIT
                            nc.sync.dma_start(
                                out=haug_ownB[layer][t2 * P:(t2 + 1) * P, :],
                                in_=stg[:, :])
                        nc.vector.tensor_copy(sdst_sb[:, t * H:(t + 1) * H],
                                              ps[:, 264:268])
                        if SPLIT2 and t == TSPLIT - 1:
                            # ---- share first half of the node table ----
                            if ABL == 4:
                                nc.sync.dma_start(
                                    out=haug_allA[layer][0, :, :],
                                    in_=haug_ownA[layer][:, :])
                            else:
                                nc.gpsimd.collective_compute(
                                    "AllGather", AluOp.bypass,
                                    replica_groups=[list(range(NCORES))],
                                    ins=[haug_ownA[layer][:, :]],
                                    outs=[haug_allA[layer][:, :, :]],
                                )
                    # ---- share second half (or, SPLIT2=0, everything) ----
                    if SPLIT2:
                        c_in, c_out = haug_ownB[layer], haug_allB[layer]
                    else:
                        c_in, c_out = haug_own[layer], haug_all[layer]
                    if ABL == 4:
                        nc.sync.dma_start(out=c_out[0, :, :], in_=c_in[:, :])
                    else:
                        nc.gpsimd.collective_compute(
                            "AllGather", AluOp.bypass,
                            replica_groups=[list(range(NCORES))],
                            ins=[c_in[:, :]],
                            outs=[c_out[:, :, :]],
                        )
                    if SPLIT2:
                        hflatA = haug_allA[layer].rearrange("c n d -> (c n) d")
                        hflatB = haug_allB[layer].rearrange("c n d -> (c n) d")
                    else:
                        hfl = haug_all[layer].rearrange("c n d -> (c n) d")
                        VH = cfg["VH"]
                        hflatA = hfl[0:VH, :]
                        hflatB = hfl[VH:NPAD, :]
                    if ABL == 1:
                        if layer == 0:
                            nc.vector.memset(xt_sb[1][:, :, :], 0.1)
                        continue

                    # ---- edge phase ----
                    offa = offb = offs = 0
                    for gi, (g0, gn, sa, sb) in enumerate(groups):
                        S = sa + sb
                        drr_t = wpool.tile([1, SLOTMAX * P], BF16, tag="drr")
                        nc.sync.dma_start(
                            out=drr_t[:, 0:S * P],
                            in_=drr_in[:, offs * P:(offs + S) * P])

                        gat = gpool.tile([P, SLOTMAX, ROW], BF16, tag="gat")
                        if ABL == 2:
                            nc.vector.memset(gat[:, :, :], 0.05)
                            nc.vector.tensor_copy(gat[:, 0:1, 0:8],
                                                  idxa_sb[:, 0:8])
                        else:
                            nc.gpsimd.dma_gather(
                                out_ap=gat[:, 0:sa, :], in_ap=hflatA[:, :],
                                idxs_ap=idxa_sb[:, offa:offa + sa * 8],
                                num_idxs=sa * P,
                                num_idxs_reg=sa * P, elem_size=ROW,
                                single_packet=False,
                                queue_num=(2 * gi) % NQ)
                            nc.gpsimd.dma_gather(
                                out_ap=gat[:, sa:S, :], in_ap=hflatB[:, :],
                                idxs_ap=idxb_sb[:, offb:offb + sb * 8],
                                num_idxs=sb * P,
                                num_idxs_reg=sb * P, elem_size=ROW,
                                single_packet=False,
                                queue_num=(2 * gi + 1) % NQ)
                        offa += sa * 8
                        offb += sb * 8

                        # transposed one-hot: ohT[n, s, e] = (n == dstrel[s,e])
                        ohT = wpool.tile([P, SLOTMAX, P], BF16, tag="ohT")
                        nch = (S + 3) // 4 if ABL != 10 else 0
                        for c in range(nch):
                            cols = min(4, S - 4 * c) * P
                            bc_ps = pspool.tile([P, 512], F32, tag="mm",
                                                name="bc_ps")
                            nc.tensor.matmul(
                                bc_ps[:, 0:cols],
                                lhsT=ones1[:, :],
                                rhs=drr_t[:, 4 * c * P:4 * c * P + cols],
                                start=True, stop=True)
                            iotac_ap = iotac[:, :]
                            nc.vector.tensor_tensor(
                                out=ohT[:, 4 * c:4 * c + cols // P, :],
                                in0=apx(iotac_ap,
                                        [iotac_ap.ap[0], [0, cols // P],
                                         [1, P]]),
                                in1=apx(bc_ps[:, :],
                                        [bc_ps[:, :].ap[0], [P, cols // P],
                                         [1, P]]),
                                op=AluOp.is_equal)

                        # per-edge s_dst via ohT matmul against own s_dst
                        sdacc = sdpool.tile([P, SLOTMAX, H], F32, tag="sdacc")
                        if ABL != 10:
                            sl = 0
                            for h_ in (0, 1):
                                for tl in range(gn):
                                    t = g0 + tl
                                    for _ in range(KA[t] if h_ == 0 else KB[t]):
                                        nc.tensor.matmul(
                                            sdacc[:, sl, :],
                                            lhsT=ohT[:, sl, :],
                                            rhs=sdst_sb[:, t * H:(t + 1) * H],
                                            start=True, stop=True)
                                        sl += 1

                        # e = lrelu(ssrc + sdst); w = exp(e)
                        ef = spool.tile([P, SLOTMAX, H], F32, tag="ef")
                        if ABL == 10:
                            nc.vector.tensor_scalar(
                                out=ef[:, 0:S, :], in0=gat[:, 0:S, 260:264],
                                scalar1=1.0, scalar2=None, op0=AluOp.mult)
                        else:
                            nc.vector.tensor_tensor(
                                out=ef[:, 0:S, :], in0=gat[:, 0:S, 260:264],
                                in1=sdacc[:, 0:S, :], op=AluOp.add)
                        efs = spool.tile([P, SLOTMAX, H], F32, tag="efs")
                        nc.vector.tensor_scalar(
                            out=efs[:, 0:S, :], in0=ef[:, 0:S, :], scalar1=0.2,
                            scalar2=None, op0=AluOp.mult)
                        nc.vector.tensor_tensor(
                            out=ef[:, 0:S, :], in0=ef[:, 0:S, :],
                            in1=efs[:, 0:S, :], op=AluOp.max)
                        wexp = spool.tile([P, SLOTMAX, H], BF16, tag="wexp")
                        nc.scalar.activation(wexp[:, 0:S, :], ef[:, 0:S, :],
                                             Act.Exp)

                        # one-hot [128e, S, 128n]
                        oh = wpool.tile([P, SLOTMAX, P], BF16, tag="oh")
                        dr = dstrel_sb[:, offs:offs + S]
                        iota_ap = iota_bf[:, :]
                        nc.vector.tensor_tensor(
                            out=oh[:, 0:S, :],
                            in0=apx(iota_ap, [iota_ap.ap[0], [0, S], [1, P]]),
                            in1=dr.to_broadcast([P, S, P]),
                            op=AluOp.is_equal)

                        # scale gathered rows by w; cols 256:260 = w itself
                        hts = wpool.tile([P, SLOTMAX, 260], BF16, tag="hts")
                        if ABL == 12:
                            nc.vector.tensor_copy(hts[:, 0:1, :],
                                                  gat[:, 0:1, 0:260])
                        else:
                            nc.vector.tensor_tensor(
                                out=hts[:, 0:S, 0:256].rearrange(
                                    "p s (h d) -> p s h d", h=H),
                                in0=gat[:, 0:S, 0:256].rearrange(
                                    "p s (h d) -> p s h d", h=H),
                                in1=wexp[:, 0:S, :].to_broadcast([P, S, H, DH]),
                                op=AluOp.mult)
                            nc.vector.tensor_copy(hts[:, 0:S, 256:260],
                                                  wexp[:, 0:S, :])

                        # scatter + normalize + BN/ELU per tile
                        aoff = 0
                        boff = sa
                        for tl in range(gn):
                            t = g0 + tl
                            slots = ([aoff + s for s in range(KA[t])] +
                                     [boff + s for s in range(KB[t])])
                            aoff += KA[t]
                            boff += KB[t]
                            if ABL == 11:
                                slots = slots[:1]
                            K_t = len(slots)
                            acc = accpool.tile([P, 260], F32, tag="acc")
                            rsrc = gat if ABL == 12 else hts
                            for j, slx in enumerate(slots):
                                nc.tensor.matmul(
                                    acc[:, :],
                                    lhsT=oh[:, slx, :],
                                    rhs=rsrc[:, slx, 0:260],
                                    start=(j == 0), stop=(j == K_t - 1),
                                )
                            tmax = spool.tile([P, H], F32, tag="tmax")
                            nc.vector.tensor_scalar(
                                out=tmax[:, :], in0=acc[:, 256:260],
                                scalar1=1e-9, scalar2=None, op0=AluOp.max)
                            rec = spool.tile([P, H], F32, tag="rec")
                            nc.vector.reciprocal(rec[:, :], tmax[:, :])
                            zsb = spool.tile([P, F], F32, tag="zsb")
                            nc.vector.tensor_tensor(
                                out=zsb[:, :].rearrange("p (h d) -> p h d",
                                                        h=H),
                                in0=acc[:, 0:256].rearrange("p (h d) -> p h d",
                                                            h=H),
                                in1=rec[:, :].to_broadcast([P, H, DH]),
                                op=AluOp.mult)
                            # transpose + BN + ELU per feature chunk
                            cls_in = spool.tile([P, FC, P], BF16, tag="clsin")
                            for fc in range(FC):
                                pst = pspool.tile([P, P], F32, tag="ptr")
                                nc.tensor.transpose(
                                    pst[:, :], zsb[:, fc * P:(fc + 1) * P],
                                    ident[:, :])
                                ybn = spool.tile([P, P], F32, tag="ybn")
                                nc.scalar.activation(
                                    ybn[:, :], pst[:, :], Act.Identity,
                                    bias=bv_sb[layer][:, fc:fc + 1],
                                    scale=gv_sb[layer][:, fc:fc + 1])
                                ey = spool.tile([P, P], F32, tag="ey")
                                nc.scalar.activation(ey[:, :], ybn[:, :],
                                                     Act.Exp)
                                nc.vector.tensor_scalar(
                                    out=ey[:, :], in0=ey[:, :], scalar1=1.0,
                                    scalar2=0.0, op0=AluOp.subtract,
                                    op1=AluOp.min)
                                nc.vector.tensor_scalar(
                                    out=ybn[:, :], in0=ybn[:, :], scalar1=0.0,
                                    scalar2=None, op0=AluOp.max)
                                if layer == 0:
                                    nc.vector.tensor_tensor(
                                        out=xt_sb[1][:, fc, t * P:(t + 1) * P],
                                        in0=ey[:, :], in1=ybn[:, :],
                                        op=AluOp.add)
                                else:
                                    nc.vector.tensor_tensor(
                                        out=cls_in[:, fc, :],
                                        in0=ey[:, :], in1=ybn[:, :],
                                        op=AluOp.add)
                            if layer == 1:
                                # fused classifier for this tile
                                cps = pspool.tile([P, 512], F32, tag="mm",
                                                  name="cps")
                                for kc in range(FC):
                                    nc.tensor.matmul(
                                        cps[:, 0:OUT],
                                        lhsT=cls_in[:, kc, :],
                                        rhs=wct_sb[:, kc, :],
                                        start=(kc == 0), stop=(kc == FC - 1),
                                    )
                                ob = spool.tile([P, OUT], F32, tag="ob")
                                nc.vector.tensor_tensor(
                                    out=ob[:, :], in0=cps[:, 0:OUT],
                                    in1=bc_sb[:, :], op=AluOp.add)
                                nc.sync.dma_start(
                                    out=out_ext[t * P:(t + 1) * P, :],
                                    in_=ob[:, :])
                        offs += S

                if ABL == 1:
                    # classifier over (garbage) layer-1 buffer, timing only
                    for t in range(NT):
                        cps = pspool.tile([P, 512], F32, tag="mm", name="cps1")
                        for kc in range(FC):
                            nc.tensor.matmul(
                                cps[:, 0:OUT],
                                lhsT=xt_sb[1][:, kc, t * P:(t + 1) * P],
                                rhs=wct_sb[:, kc, :],
                                start=(kc == 0), stop=(kc == FC - 1),
                            )
                        ob = spool.tile([P, OUT], F32, tag="ob")
                        nc.vector.tensor_tensor(out=ob[:, :], in0=cps[:, 0:OUT],
                                                in1=bc_sb[:, :], op=AluOp.add)
                        nc.sync.dma_start(out=out_ext[t * P:(t + 1) * P, :],
                                          in_=ob[:, :])

    nc.compile()
    return nc


# --------------------------------------------------------------------------
# entry point
# --------------------------------------------------------------------------

def kernel(x, edge_index, W1, a_src1, a_dst1, bn1_gamma, bn1_beta, bn1_mean,
           bn1_var, W2, a_src2, a_dst2, bn2_gamma, bn2_beta, bn2_mean, bn2_var,
           Wc, bc, _cfg=None, _run_kwargs=None, _bench=0):
    cfg = dict(CFG)
    if _cfg:
        cfg.update(_cfg)
    N, F, OUT = cfg["N"], cfg["F"], cfg["OUT"]
    FC = F // P

    if cfg.get("PERM", 1):
        pos = balance_perm(cfg, edge_index)
        edge_index = pos[np.asarray(edge_index, np.int64)]
    else:
        pos = None
    groups, KA, KB, TOT, per_core, NB, NT = prep_edges(cfg, edge_index)
    nc = build_kernel(cfg, groups, KA, KB, TOT, NB, NT)

    wfull1 = _wfull(W1, a_src1, a_dst1)
    wfull2 = _wfull(W2, a_src2, a_dst2)
    wct = np.ascontiguousarray(np.asarray(Wc, np.float64).T).astype(
        ml_dtypes.bfloat16)
    g1, b1 = _bn_consts(bn1_gamma, bn1_beta, bn1_mean, bn1_var)
    g2, b2 = _bn_consts(bn2_gamma, bn2_beta, bn2_mean, bn2_var)
    bc_rep = np.tile(np.asarray(bc, np.float32)[None, :], (P, 1))

    xpad = np.zeros((NB * NCORES, F), np.float32)
    if pos is not None:
        xpad[pos[:N]] = np.asarray(x, np.float32)
    else:
        xpad[:N] = np.asarray(x, np.float32)
    xt = np.ascontiguousarray(xpad.T).astype(ml_dtypes.bfloat16)  # [F, NPAD]

    in_maps = []
    for k in range(NCORES):
        xk = xt[:, k * NB:(k + 1) * NB].reshape(FC, P, NB)
        in_maps.append(dict(
            xt=np.ascontiguousarray(xk),
            wfull1=np.ascontiguousarray(wfull1.reshape(FC, P, 268)),
            wfull2=np.ascontiguousarray(wfull2.reshape(FC, P, 268)),
            wct=np.ascontiguousarray(wct.reshape(FC, P, OUT)),
            gvec1=g1, bvec1=b1, gvec2=g2, bvec2=b2, bc_rep=bc_rep,
            idxa=per_core[k]["idxa"], idxb=per_core[k]["idxb"],
            dstrel=per_core[k]["dstrel"], dstrelr=per_core[k]["dstrelr"],
            ident=np.eye(P, dtype=np.float32),
            iotarow=np.tile(np.arange(P, dtype=np.float32)[None, :],
                            (P, 1)).astype(ml_dtypes.bfloat16),
            iotacol=np.ascontiguousarray(
                np.tile(np.arange(P, dtype=np.float32)[:, None], (1, P))),
            ones1=np.ones((1, P), np.float32).astype(ml_dtypes.bfloat16),
        ))

    res = run_bass_kernel_spmd(nc, in_maps, list(range(NCORES)),
                               **(_run_kwargs or {}))
    out = np.concatenate([res.results[k]["out"] for k in range(NCORES)], axis=0)
    if pos is not None:
        out = out[pos[:N]]
    out = out[:N].astype(np.float32)
    if _bench:
        ns = _bench_pjrt(nc, in_maps, _bench)
        return out, ns
    if _run_kwargs is not None:
        return out, res
    return out


def _bench_pjrt(nc, in_maps, iters):
    """Median per-iteration wall time (ns) of the NEFF execution via PJRT,
    device-resident inputs, back-to-back async dispatch."""
    import time
    import jax
    import jax.numpy as jnp
    from jax.sharding import Mesh, PartitionSpec
    from jax.experimental.shard_map import shard_map
    from concourse import bass2jax
    from concourse.bass2jax import _bass_exec_p, partition_id_tensor
    import concourse.mybir as mybir

    n_cores = len(in_maps)
    partition_name = nc.partition_id_tensor.name if nc.partition_id_tensor else None
    in_names, out_names, out_avals, zero_outs = [], [], [], []
    for alloc in nc.m.functions[0].allocations:
        if not isinstance(alloc, mybir.MemoryLocationSet):
            continue
        name = alloc.memorylocations[0].name
        if alloc.kind == "ExternalInput":
            if name != partition_name:
                in_names.append(name)
        elif alloc.kind == "ExternalOutput":
            shape = list(alloc.tensor_shape)
            dt = mybir.dt.np(alloc.dtype)
            out_avals.append(jax.core.ShapedArray(shape, dt))
            out_names.append(name)
            zero_outs.append(np.zeros(shape, dt))
    n_params = len(in_names)
    n_outs = len(out_names)
    in_names.extend(out_names)
    if partition_name is not None:
        in_names.append(partition_name)

    def _body(*args):
        operands = list(args)
        if partition_name is not None:
            operands.append(partition_id_tensor())
        return tuple(_bass_exec_p.bind(
            *operands, out_avals=tuple(out_avals), in_names=tuple(in_names),
            out_names=tuple(out_names), lowering_input_output_aliases=(),
            sim_require_finite=True, sim_require_nnan=True, nc=nc))

    devices = jax.devices()[:n_cores]
    mesh = Mesh(np.asarray(devices), ("core",))
    sharded = jax.jit(
        shard_map(_body, mesh=mesh,
                  in_specs=(PartitionSpec("core"),) * (n_params + n_outs),
                  out_specs=(PartitionSpec("core"),) * n_outs,
                  check_rep=False),
        donate_argnums=(), keep_unused=True)
    per_core = [[np.asarray(m[name]) for name in in_names[:n_params]]
                for m in in_maps]
    concat_in = [np.concatenate([per_core[c][i] for c in range(n_cores)], axis=0)
                 for i in range(n_params)]
    from jax.sharding import NamedSharding
    sh = NamedSharding(mesh, PartitionSpec("core"))
    dev_in = [jax.device_put(a, sh) for a in concat_in]
    zshapes = [(n_cores * z.shape[0], *z.shape[1:]) for z in zero_outs]
    zdtypes = [z.dtype for z in zero_outs]

    dev_zeros = [jax.device_put(np.zeros(s_, d_), sh)
                 for s_, d_ in zip(zshapes, zdtypes)]

    def one_iter():
        return sharded(*dev_in, *dev_zeros)

    jax.block_until_ready(one_iter())
    times = []
    for _ in range(5):
        t0 = time.perf_counter()
        outs = [one_iter() for _ in range(iters)]
        jax.block_until_ready(outs[-1])
        times.append((time.perf_counter() - t0) / iters * 1e9)
    return min(times)
